# revision 1
# baseline (speedup 1.0000x reference)
"""Trainium2 Bass kernel for nn_DecoderCrossAttention.

Reference computation (per voxel v, batch b):
    q = Wq x_v + bq                        (x = decoder_features, [C])
    k_j = Wk y_jv + bk, v_j = Wv y_jv + bv (y = skip features, COND=4 frames)
    s_j[h] = <q_h, k_jh> / sqrt(DH)        (NH=8 heads of DH=16)
    attn = softmax_j(s)                    (over the 4 conditioning frames)
    o = Wo (sum_j attn_j * v_j) + bo + x_v
    out = GroupNorm8(o) * gamma + beta     (stats over (C/G, H, W, D) per batch)

Strategy (8 NeuronCores, data-parallel over H):
  * Each core gets H-slice of 4 planes: 2*4*32*32 = 8192 voxels.
  * Feature-major layout [C=128 partitions, voxels in free dim], 512-voxel tiles.
  * x/y/Wq/Wk/Wv in bf16 (host-cast, host-transposed weights): halves input
    DMA; rel-err gate is 2e-2, measured ~2.4e-3.
  * Per-head score reduction (sum over the 16 channels of a head) and the
    softmax broadcast (8 head rows -> 128 channels) are PE matmuls against
    0/1 masks built in-kernel with iota+compare.
  * Softmax over only 4 logits, inputs are bounded => no max subtraction.
  * E~ = exp(s)*recip(Z): exp on Act, recip on DVE, the product on Pool.
  * qk uses a stride-0 broadcast AP of q over the 2-cond pair (the DVE
    allows only ONE PSUM operand per op - NCC_IBVF027).
  * attn*V products (per-cond, through two ping-ponged 1-bank PSUM pools
    so the PE broadcast hides under the previous cond's DVE multiply) feed
    4 accumulating output-projection matmuls, the residual rides in as a
    5th identity matmul, and the Act engine evacuates psO with
    bias+per-channel sums (accum_out); ssq via Act Square.  DVE carries
    only qk/avm/recip in steady state (~99% occupied).
  * GroupNorm is global: per-channel sum/sumsq AllGather (15us fixed vs
    28us for AllReduce in the cost model) + local reduce; a
    zero-contribution stt gates the batch-0 finalization into the batch-1
    AllGather window.  Group stats are reduced+broadcast+meaned in ONE
    pre-scaled [C,C] group-mask matmul, variance via sqrt(scale=-1,
    bias=eps), and the rescale is done in-place on out_acc split 4/2/2
    over DVE (2x all-SBUF mode) / Act (Identity scale+bias) / Pool.
  * Constants are packed into 3 DMAs (wqkv, wo, vec6) and y is fetched
    per-cond with a 2-tile prefetch: first matmul fires at ~5us.

The walrus build here accepts only ONE sync wait per instruction; Tile
attaches many.  split_waits() hoists extras onto standalone EventSemaphore
instructions post-scheduling.
"""

import sys

if "/opt/trn_rl_repo" not in sys.path:
    sys.path.insert(0, "/opt/trn_rl_repo")

import numpy as np

B, COND, C, H, W, D = 2, 4, 128, 32, 32, 32
NH, DH, G = 8, 16, 8
EPS = 1e-5
NCORES = 8
HS = H // NCORES          # 4 H-planes per core
NVOX = HS * W * D         # 4096 voxels per batch per core
NT = 512                  # voxels per tile
NTILES = NVOX // NT       # 8 tiles per batch
N_GROUP = (C // G) * H * W * D   # elements per (batch, group) for GN stats

_CACHE = {}


def _split_waits(nc):
    """Hoist extra sync waits onto standalone EventSemaphore instructions."""
    from concourse import mybir
    import bass_rust

    n_split = 0
    for func in nc.m.functions:
        for blk in func.blocks:
            new_list = []
            changed = False
            for inst in blk.instructions:
                si = inst.sync_info
                waits = list(si.on_wait) if si is not None else []
                if len(waits) > 1:
                    changed = True
                    for w in waits[:-1]:
                        ev = mybir.InstEventSemaphore(
                            name=f"wsplit-{nc.next_id()}", ins=[], outs=[]
                        )
                        ev.engine = inst.engine
                        ev.sync_info = bass_rust.SyncInfo(on_wait=[w], on_update=[])
                        new_list.append(ev)
                        n_split += 1
                    inst.sync_info = bass_rust.SyncInfo(
                        on_wait=[waits[-1]], on_update=list(si.on_update)
                    )
                new_list.append(inst)
            if changed:
                blk.instructions = new_list
    return n_split


def _build(n_reps=1):
    import concourse.bass as bass
    import concourse.tile as tile
    from concourse import mybir
    from contextlib import ExitStack

    dt = mybir.dt
    f32 = dt.float32
    f32r = dt.float32r
    i32 = dt.int32
    Alu = mybir.AluOpType
    Act = mybir.ActivationFunctionType
    ts = bass.ts

    bf16 = dt.bfloat16

    nc = bass.Bass("TRN2", target_bir_lowering=False, debug=False,
                   num_devices=NCORES)
    x_io = nc.dram_tensor("x", [B, C, NVOX], bf16, kind="ExternalInput").ap()
    y_io = nc.dram_tensor("y", [B, COND, C, NVOX], bf16, kind="ExternalInput").ap()
    # constants packed into 3 tensors: each dma_start costs ~0.6us of HWDGE
    # issue time, so 10 separate loads would delay the first x/y input DMAs
    wqkv_io = nc.dram_tensor("wqkv", [C, 3 * C], bf16, kind="ExternalInput").ap()
    wo_io = nc.dram_tensor("wo", [C, C], f32r, kind="ExternalInput").ap()
    vec6_io = nc.dram_tensor("vec6", [C, 6], f32, kind="ExternalInput").ap()
    out_io = nc.dram_tensor("out", [B, C, NVOX], f32, kind="ExternalOutput").ap()

    def mm(out, lhsT, rhs, start=True, stop=True):
        nc.tensor.matmul(out, lhsT=lhsT, rhs=rhs, start=start, stop=stop)

    with tile.TileContext(nc) as tc, ExitStack() as ctx:
        # ---------------- constants / weights / masks -------------------
        const = ctx.enter_context(tc.tile_pool(name="const", bufs=1))
        dram = ctx.enter_context(tc.tile_pool(name="dram", bufs=1, space="DRAM"))

        # constants first, on the Activation HWDGE queue, packed: 3 issues
        wqkv = const.tile([C, 3 * C], bf16, tag="wqkv")
        nc.scalar.dma_start(wqkv[:], wqkv_io[:])
        wo_t = const.tile([C, C], f32r, tag="wT_wo")
        nc.scalar.dma_start(wo_t[:], wo_io[:])
        vec6 = const.tile([C, 6], f32, tag="vec6")
        nc.scalar.dma_start(vec6[:], vec6_io[:])
        wT = {"wq": wqkv[:, 0:C], "wk": wqkv[:, C:2 * C],
              "wv": wqkv[:, 2 * C:3 * C], "wo": wo_t[:]}
        vecs = {name: vec6[:, i:i + 1] for i, name in
                enumerate(("bq", "bk", "bv", "bo", "gamma", "beta"))}

        # --- masks via iota + compare (int32), cast to f32
        with tc.tile_pool(name="setup", bufs=1) as setup:
            def icast(dst_ap, src_ap):
                nc.vector.tensor_copy(dst_ap, src_ap)

            # partition-index and free-index helpers
            p128 = setup.tile([C, C], i32, tag="p128")
            nc.gpsimd.iota(p128[:], pattern=[[0, C]], base=0, channel_multiplier=1)
            f128 = setup.tile([C, C], i32, tag="f128")
            nc.gpsimd.iota(f128[:], pattern=[[1, C]], base=0, channel_multiplier=0)
            hc128 = setup.tile([C, C], i32, tag="hc128")
            nc.vector.tensor_scalar(hc128[:], p128[:], 4, None,
                                    Alu.arith_shift_right)
            tmpi = setup.tile([C, C], i32, tag="tmpi")

            # identity [128,128] (bf16, for the residual pass-through matmul)
            ident = const.tile([C, C], bf16, tag="ident")
            nc.vector.tensor_tensor(tmpi[:], f128[:], p128[:], Alu.is_equal)
            icast(ident[:], tmpi[:])

            # mask32 [128, 4*32]: col 32j+m ; 1 iff (m - 8j) == c//16
            jm = setup.tile([C, C], i32, tag="jm")
            nc.gpsimd.iota(jm[:].rearrange("p (j m) -> p j m", j=4),
                           pattern=[[-8, 4], [1, 32]], base=0,
                           channel_multiplier=0)
            mask32 = const.tile([C, C], bf16, tag="mask32")
            nc.vector.tensor_tensor(tmpi[:], jm[:], hc128[:], Alu.is_equal)
            icast(mask32[:], tmpi[:])

            # lhsT32 [32,32]: 1 iff p%8 == m%8  (Z replication matmul)
            p32 = setup.tile([32, 32], i32, tag="p32")
            nc.gpsimd.iota(p32[:], pattern=[[0, 32]], base=0, channel_multiplier=1)
            pm32 = setup.tile([32, 32], i32, tag="pm32")
            nc.vector.tensor_scalar(pm32[:], p32[:], 3, 3,
                                    Alu.arith_shift_right, Alu.arith_shift_left)
            t32 = setup.tile([32, 32], i32, tag="t32")
            nc.vector.tensor_tensor(t32[:], p32[:], pm32[:], Alu.subtract)
            fm32 = setup.tile([32, 32], i32, tag="fm32")
            nc.gpsimd.iota(fm32[:].rearrange("p (j m) -> p j m", j=4),
                           pattern=[[0, 4], [1, 8]], base=0, channel_multiplier=0)
            e32 = setup.tile([32, 32], i32, tag="e32")
            nc.vector.tensor_tensor(e32[:], fm32[:], t32[:], Alu.is_equal)
            lhsT32 = const.tile([32, 32], f32r, tag="lhsT32")
            icast(lhsT32[:], e32[:])

            # maskb [32, 4*128]: col 128j+c ; 1 iff (p - 8j) == c//16
            pj = setup.tile([32, 4 * C], i32, tag="pj")
            nc.gpsimd.iota(pj[:].rearrange("p (j c) -> p j c", j=4),
                           pattern=[[-8, 4], [0, C]], base=0,
                           channel_multiplier=1)
            fc = setup.tile([32, 4 * C], i32, tag="fc")
            nc.gpsimd.iota(fc[:].rearrange("p (j c) -> p j c", j=4),
                           pattern=[[0, 4], [1, C]], base=0, channel_multiplier=0)
            nc.vector.tensor_scalar(fc[:], fc[:], 4, None, Alu.arith_shift_right)
            eb = setup.tile([32, 4 * C], i32, tag="eb")
            nc.vector.tensor_tensor(eb[:], pj[:], fc[:], Alu.is_equal)
            maskb = const.tile([32, 4 * C], bf16, tag="maskb")
            icast(maskb[:], eb[:])

            # ggmask [128, 128]: 1/N_GROUP iff p//16 == c//16  (GN group sum,
            # fused reduce+broadcast+mean: psGB = ggmask.T @ stats gives the
            # group means [mean, E[x^2]] directly at channel layout)
            fg = setup.tile([C, C], i32, tag="fg")
            nc.vector.tensor_scalar(fg[:], f128[:], 4, None,
                                    Alu.arith_shift_right)
            egg = setup.tile([C, C], i32, tag="egg")
            nc.vector.tensor_tensor(egg[:], fg[:], hc128[:], Alu.is_equal)
            ggmask = const.tile([C, C], f32, tag="ggmask")
            icast(ggmask[:], egg[:])
            nc.vector.tensor_scalar(ggmask[:], ggmask[:], 1.0 / N_GROUP, None,
                                    Alu.mult)
            eps_t = const.tile([C, 1], f32, tag="eps_t")
            nc.vector.memset(eps_t[:], EPS)

        # ---------------- main pipeline ---------------------------------
        per_rep_pools = dict(
            xres=ctx.enter_context(tc.tile_pool(name="xres", bufs=2)),
            ypool=ctx.enter_context(tc.tile_pool(name="ypool", bufs=2)),
            sb=ctx.enter_context(tc.tile_pool(name="sb", bufs=2)),
            big=ctx.enter_context(tc.tile_pool(name="bigsb", bufs=2)),
            # bufs=2 so rep r+1's accumulation/stats don't serialize behind
            # rep r's finalization (cross-rep overlap; no-op for n_reps=1)
            opool=ctx.enter_context(tc.tile_pool(name="opool", bufs=2)),
            stats=ctx.enter_context(tc.tile_pool(name="stats", bufs=2)),
            ps_kb=ctx.enter_context(tc.tile_pool(name="ps_kb", bufs=1, space="PSUM")),
            ps_bb1=ctx.enter_context(tc.tile_pool(name="ps_bb1", bufs=1, space="PSUM")),
            ps_bb2=ctx.enter_context(tc.tile_pool(name="ps_bb2", bufs=1, space="PSUM")),
            ps_q=ctx.enter_context(tc.tile_pool(name="ps_q", bufs=1, space="PSUM")),
            ps_v=ctx.enter_context(tc.tile_pool(name="ps_v", bufs=1, space="PSUM")),
            ps_s=ctx.enter_context(tc.tile_pool(name="ps_s", bufs=1, space="PSUM")),
            ps_o=ctx.enter_context(tc.tile_pool(name="ps_o", bufs=1, space="PSUM")),
        )

        for rep in range(n_reps):
            p = per_rep_pools
            out_acc = p["opool"].tile([C, B * NVOX], f32, tag="out_acc")
            sums = p["stats"].tile([C, B * NTILES], f32, tag="sums")
            ssqs = p["stats"].tile([C, B * NTILES], f32, tag="ssqs")
            dump = p["stats"].tile([C, NT], f32, tag="dump")

            tiles = [(b, t) for b in range(B) for t in range(NTILES)]
            xres_b = {}
            ychunk_state = {}
            fstate = {}
            sstate = {}

            def load_x(b):
                if b in xres_b:
                    return
                xr = p["xres"].tile([C, NVOX], bf16, tag="xres")
                nc.sync.dma_start(xr[:], x_io[b])
                xres_b[b] = xr

            def load_ychunk(b, ci):
                if (b, ci) in ychunk_state:
                    return
                yc = p["ypool"].tile([C, COND * 4 * NT], bf16, tag="ychunk")
                ysrc = y_io[b].rearrange("j c v -> c j v")
                # per-cond DMAs: K-proj for cond j only waits on its slice
                for j in range(COND):
                    nc.sync.dma_start(
                        yc[:, j * 4 * NT: (j + 1) * 4 * NT],
                        ysrc[:, bass.ts(j, 1), bass.ts(ci, 4 * NT)]
                        .rearrange("p j v -> p (j v)"),
                    )
                ychunk_state[(b, ci)] = yc

            def front1(k):
                """DMAs, Q proj, K h0 proj, V proj+copies."""
                b, t = tiles[k]
                load_x(b)
                load_ychunk(b, t // 4)
                ychunk = ychunk_state[(b, t // 4)]
                yj_of = lambda j: ychunk[:, j * 4 * NT + (t % 4) * NT:
                                         j * 4 * NT + (t % 4 + 1) * NT]
                xt = xres_b[b][:, ts(t, NT)]
                psQ = p["ps_q"].tile([C, NT], f32, tag="psq")
                mm(psQ[:], wT["wq"], xt)
                qsb = p["sb"].tile([C, NT], f32, tag="qsb")
                nc.scalar.activation(qsb[:], psQ[:], Act.Identity,
                                     bias=vecs["bq"])
                psKB0 = p["ps_kb"].tile([C, 2 * NT], f32, tag="kb")
                for j in (0, 1):
                    mm(psKB0[:, ts(j, NT)], wT["wk"], yj_of(j))
                vbig = p["big"].tile([C, COND * NT], f32, tag="vbig")
                for j in range(COND):
                    psV = p["ps_v"].tile([C, NT], f32, tag="psv")
                    mm(psV[:], wT["wv"], yj_of(j))
                    nc.scalar.activation(vbig[:, ts(j, NT)], psV[:],
                                         Act.Identity, bias=vecs["bv"])
                qkbig = p["big"].tile([C, COND * NT], bf16, tag="qkbig")
                psS = p["ps_s"].tile([32, NT], f32, tag="pss")
                fstate[k] = (psS, vbig, xt, qsb, qkbig, yj_of, psKB0)
                # prefetch the next y chunk / next batch's inputs during the
                # current chunk's idle DMA time (avoids a burst at t%4==0 and
                # at the batch boundary, where stores also compete)
                if t % 4 == 2:
                    if t // 4 + 1 < NTILES // 4:
                        load_ychunk(b, t // 4 + 1)
                    elif b + 1 < B:
                        load_x(b + 1)
                        load_ychunk(b + 1, 0)

            def front_qk(k, h):
                """QK mul half h + its score matmuls (+ K h1 projections).

                qk = (k + bk) * q via a stride-0 broadcast of qsb over the
                two conds (one PSUM operand only: HW limit NCC_IBVF027).
                """
                psS, vbig, xt, qsb, qkbig, yj_of, psKB = fstate[k]
                qrep = qsb[:].unsqueeze(1).broadcast_to([C, 2, NT])
                nc.vector.scalar_tensor_tensor(
                    qkbig[:, ts(h, 2 * NT)].rearrange("p (j v) -> p j v", j=2),
                    psKB[:].rearrange("p (j v) -> p j v", j=2),
                    vecs["bk"], qrep, Alu.add, Alu.mult)
                for j in (2 * h, 2 * h + 1):
                    mm(psS[:], mask32[:, ts(j, 32)], qkbig[:, ts(j, NT)],
                       start=(j == 0), stop=(j == COND - 1))
                if h == 0:
                    psKB1 = p["ps_kb"].tile([C, 2 * NT], f32, tag="kb")
                    for j in (2, 3):
                        mm(psKB1[:, ts(j - 2, NT)], wT["wk"], yj_of(j))
                    fstate[k] = (psS, vbig, xt, qsb, qkbig, yj_of, psKB1)

            def soft(k):
                """exp, Z-matmul, reciprocal, E~ = E * (1/Z)."""
                b, t = tiles[k]
                psS, vbig, xt, qsb, qkbig, yj_of, _ = fstate.pop(k)
                esb = p["sb"].tile([32, NT], f32r, tag="esb")
                nc.scalar.activation(esb[:], psS[:], Act.Exp, scale=0.25)
                # psZ lives in the ps_s pool: its WAR (next tile's scores)
                # sits later in the chain than ps_q's (next tile's Q-proj).
                psZ = p["ps_s"].tile([32, NT], f32, tag="pss")
                mm(psZ[:], lhsT32[:], esb[:])
                rsb = p["sb"].tile([32, NT], f32, tag="rsb")
                nc.vector.reciprocal(rsb[:], psZ[:])
                etsb = p["sb"].tile([32, NT], bf16, tag="etsb")
                nc.gpsimd.tensor_tensor(etsb[:], esb[:].bitcast(f32),
                                        rsb[:], Alu.mult)
                sstate[k] = (etsb, vbig, xt)

            def back_cond(k, j, wbig):
                """Broadcast matmul + attn*V multiply for one cond.

                Per-cond with two ping-ponged 1-bank PSUM pools: bcast j+2
                only waits on avm j, so the PE bcast hides under the DVE
                multiply of the previous cond (a shared 2-bank buffer cost
                736ns of DVE idle per tile here).
                """
                etsb, vbig, xt = sstate[k]
                pool = p["ps_bb1"] if j % 2 == 0 else p["ps_bb2"]
                psB = pool.tile([C, NT], f32, tag="bb")
                mm(psB[:], maskb[:, ts(j, C)], etsb[:])
                nc.vector.tensor_tensor(
                    wbig[:, ts(j, NT)], psB[:],
                    vbig[:, ts(j, NT)], Alu.mult)

            def back_out(k, wbig):
                b, t = tiles[k]
                col = b * NTILES + t
                etsb, vbig, xt = sstate.pop(k)
                psO = p["ps_o"].tile([C, NT], f32, tag="pso")
                for j in range(COND):
                    mm(psO[:], wT["wo"], wbig[:, ts(j, NT)],
                       start=(j == 0), stop=False)
                # residual folded into the accumulation group: psO += I @ x
                mm(psO[:], ident[:], xt, start=False, stop=True)
                outt = out_acc[:, col * NT: (col + 1) * NT]
                nc.scalar.activation(
                    outt, psO[:], Act.Identity, bias=vecs["bo"],
                    accum_out=sums[:, col: col + 1])
                nc.scalar.activation(
                    dump[:], outt, Act.Square,
                    accum_out=ssqs[:, col: col + 1])

            cc_state = {}
            cc_sb = {}

            def gn_pre(b):
                """Reduce per-channel stats and launch the AllGather."""
                ccsb = p["stats"].tile([C, 2], f32, tag=f"ccsb{b}")
                nc.vector.reduce_sum(ccsb[:, 0:1],
                                     sums[:, b * NTILES:(b + 1) * NTILES],
                                     axis=mybir.AxisListType.X)
                nc.vector.reduce_sum(ccsb[:, 1:2],
                                     ssqs[:, b * NTILES:(b + 1) * NTILES],
                                     axis=mybir.AxisListType.X)
                cc_in = dram.tile([C, 2], f32, tag=f"cc_in{b}")
                cc_out = dram.tile([NCORES, C, 2], f32, tag=f"cc_out{b}")
                # Act HWDGE queue: must not sit behind bulk stores on SP
                nc.scalar.dma_start(cc_in[:], ccsb[:])
                nc.gpsimd.collective_compute(
                    "AllGather", Alu.bypass,
                    replica_groups=[list(range(NCORES))],
                    ins=[cc_in.opt()], outs=[cc_out.opt()])
                cc_state[b] = cc_out
                cc_sb[b] = ccsb

            def gn_post(b):
                """Stats -> per-channel affine -> rescale out_acc -> store."""
                cc_out = cc_state.pop(b)
                # gather the 8 cores' [C,2] partials: gs16[:, 0:8]=sums,
                # gs16[:, 8:16]=ssqs (s-major so the reduces are contiguous)
                gs16 = p["stats"].tile([C, 16], f32, tag=f"gs16_{b}")
                nc.sync.dma_start(
                    gs16[:].rearrange("p (s n) -> p s n", n=NCORES),
                    cc_out[:].rearrange("n p s -> p s n"))
                gsb = p["stats"].tile([C, 2], f32, tag=f"gsb{b}")
                nc.vector.reduce_sum(gsb[:, 0:1], gs16[:, 0:NCORES],
                                     axis=mybir.AxisListType.X)
                nc.vector.reduce_sum(gsb[:, 1:2], gs16[:, NCORES:2 * NCORES],
                                     axis=mybir.AxisListType.X)
                # fused group reduce+broadcast+mean: psGB = [mean, E[x^2]]
                psGB = p["ps_q"].tile([C, 2], f32, tag="psq")
                nc.tensor.matmul(psGB[:], lhsT=ggmask[:], rhs=gsb[:],
                                 start=True, stop=True)
                msb = p["stats"].tile([C, 2], f32, tag=f"msb{b}")
                nc.vector.tensor_copy(msb[:], psGB[:])
                # negvar = mean^2 - E[x^2]; sqrt runs with scale=-1, +eps
                nvar = p["stats"].tile([C, 1], f32, tag=f"nvar{b}")
                nc.vector.scalar_tensor_tensor(
                    nvar[:], msb[:, 0:1], msb[:, 0:1], msb[:, 1:2],
                    Alu.mult, Alu.subtract)
                sstd = p["stats"].tile([C, 1], f32, tag=f"sstd{b}")
                nc.scalar.activation(sstd[:], nvar[:], Act.Sqrt,
                                     bias=eps_t[:], scale=-1.0)
                rstd = p["stats"].tile([C, 1], f32, tag=f"rstd{b}")
                nc.vector.reciprocal(rstd[:], sstd[:])
                scale_b = p["stats"].tile([C, 1], f32, tag=f"scale{b}")
                nc.vector.tensor_tensor(scale_b[:], rstd[:],
                                        vecs["gamma"], Alu.mult)
                gate = cc_sb.get(1 - b) if b == 0 else None
                if gate is not None:
                    # 0*ccsb(b1) + scale: a zero-contribution data dependency
                    # that stops the scheduler hoisting b0's rescale+stores
                    # out of the b1 AllGather window they are meant to fill
                    gated = p["stats"].tile([C, 1], f32, tag=f"gsc{b}")
                    nc.vector.scalar_tensor_tensor(
                        gated[:], gate[:, 0:1], 0.0, scale_b[:],
                        Alu.mult, Alu.add)
                    scale_b = gated
                negb_b = p["stats"].tile([C, 2], f32, tag=f"negb{b}")
                nc.vector.scalar_tensor_tensor(
                    negb_b[:, 0:1], msb[:, 0:1], scale_b[:],
                    vecs["beta"], Alu.mult, Alu.subtract)
                nc.vector.tensor_scalar(negb_b[:, 1:2], negb_b[:, 0:1],
                                        -1.0, None, Alu.mult)
                # rescale split over three engines (DVE 2x / Act / Pool),
                # storing each tile as soon as it is rescaled
                for t in range(NTILES):
                    src = out_acc[:, (b * NTILES + t) * NT:
                                  (b * NTILES + t + 1) * NT]
                    if t % 4 == 3:
                        nc.gpsimd.tensor_scalar(
                            src, src,
                            scale_b[:], negb_b[:, 0:1], Alu.mult, Alu.subtract)
                    elif t % 4 == 1:
                        nc.scalar.activation(src, src, Act.Identity,
                                             scale=scale_b[:],
                                             bias=negb_b[:, 1:2])
                    else:
                        nc.vector.tensor_scalar(
                            src, src,
                            scale_b[:], negb_b[:, 0:1], Alu.mult, Alu.subtract)
                    nc.sync.dma_start(out_io[b][:, ts(t, NT)], src)

            NK = len(tiles)
            for k in range(NK + 2):
                if 1 <= k <= NK:
                    soft(k - 1)
                if k >= 2:
                    wbig = p["big"].tile([C, COND * NT], f32r, tag="qkbig")
                    back_cond(k - 2, 0, wbig)
                    back_cond(k - 2, 1, wbig)
                if k < NK:
                    front1(k)
                    front_qk(k, 0)
                if k >= 2:
                    back_cond(k - 2, 2, wbig)
                    back_cond(k - 2, 3, wbig)
                if k < NK:
                    front_qk(k, 1)
                if k >= 2:
                    back_out(k - 2, wbig)
                    bdone, tdone = tiles[k - 2]
                    if tdone == NTILES - 1:
                        gn_pre(bdone)
            # Both finalizations after the loop: gn_post(0) depends only on
            # the (long-finished) b0 AllGather, so its rescale+stores fill
            # the b1 AllGather latency.
            for b in range(B):
                gn_post(b)


    _split_waits(nc)
    return nc


def _shard_inputs(inputs):
    import ml_dtypes
    bf16 = ml_dtypes.bfloat16
    x = np.asarray(inputs["decoder_features"], np.float32).astype(bf16)
    y = np.asarray(inputs["skip_connection_features"], np.float32).astype(bf16)

    def wT(name, dtype):
        w = np.asarray(inputs[name], np.float32)
        return np.ascontiguousarray(w.T).astype(dtype)

    wqkv = np.concatenate([wT("w_q", bf16), wT("w_k", bf16),
                           wT("w_v", bf16)], axis=1)
    vec6 = np.stack([np.asarray(inputs[n], np.float32) for n in
                     ("b_q", "b_k", "b_v", "b_o", "gn_gamma", "gn_beta")],
                    axis=1)
    base = {
        "wqkv": np.ascontiguousarray(wqkv),
        "wo": wT("w_o", np.float32),
        "vec6": np.ascontiguousarray(vec6),
    }
    in_maps = []
    for ci in range(NCORES):
        sl = slice(HS * ci, HS * (ci + 1))
        im = dict(base)
        im["x"] = np.ascontiguousarray(x[:, :, sl]).reshape(B, C, NVOX)
        im["y"] = np.ascontiguousarray(y[:, :, :, sl]).reshape(B, COND, C, NVOX)
        in_maps.append(im)
    return in_maps


class _Runner:
    """Persistent PJRT runner: trace/compile once, execute many times.

    Mirrors concourse.bass2jax.run_bass_via_pjrt's multi-core branch but
    keeps the jitted shard_map callable alive so repeat calls skip
    re-tracing and NEFF recompilation.
    """

    def __init__(self, nc, donate=True):
        import jax
        from jax.sharding import Mesh, PartitionSpec
        from jax.experimental.shard_map import shard_map
        from concourse import bass2jax, mybir

        bass2jax.install_neuronx_cc_hook()
        assert nc.dbg_addr is None
        partition_name = (nc.partition_id_tensor.name
                          if nc.partition_id_tensor else None)
        in_names, out_names, out_avals, zero_outs = [], [], [], []
        for alloc in nc.m.functions[0].allocations:
            if not isinstance(alloc, mybir.MemoryLocationSet):
                continue
            name = alloc.memorylocations[0].name
            if alloc.kind == "ExternalInput":
                if name != partition_name:
                    in_names.append(name)
            elif alloc.kind == "ExternalOutput":
                out_names.append(name)
                shape = tuple(alloc.tensor_shape)
                dtype = mybir.dt.np(alloc.dtype)
                out_avals.append(jax.core.ShapedArray(shape, dtype))
                zero_outs.append(np.zeros(shape, dtype))
        n_params = len(in_names)
        n_outs = len(out_avals)
        in_names.extend(out_names)
        if partition_name is not None:
            in_names.append(partition_name)
        donate_idx = tuple(range(n_params, n_params + n_outs)) if donate else ()

        def _body(*args):
            operands = list(args)
            if partition_name is not None:
                operands.append(bass2jax.partition_id_tensor())
            outs = bass2jax._bass_exec_p.bind(
                *operands,
                out_avals=tuple(out_avals),
                in_names=tuple(in_names),
                out_names=tuple(out_names),
                lowering_input_output_aliases=(),
                sim_require_finite=True,
                sim_require_nnan=True,
                nc=nc,
            )
            return tuple(outs)

        devices = jax.devices()[:NCORES]
        mesh = Mesh(np.asarray(devices), ("core",))
        in_specs = (PartitionSpec("core"),) * (n_params + n_outs)
        out_specs = (PartitionSpec("core"),) * n_outs
        self._fn = jax.jit(
            shard_map(_body, mesh=mesh, in_specs=in_specs,
                      out_specs=out_specs, check_rep=False),
            donate_argnums=donate_idx, keep_unused=True)
        self._in_names = in_names[:n_params]
        self._out_names = out_names
        self._out_avals = out_avals
        self._zero_outs = zero_outs
        self._jax = jax

    def __call__(self, in_maps):
        concat_in = [
            np.concatenate([np.asarray(m[name]) for m in in_maps], axis=0)
            for name in self._in_names
        ]
        concat_zeros = [
            np.zeros((NCORES * z.shape[0], *z.shape[1:]), z.dtype)
            for z in self._zero_outs
        ]
        out_arrs = self._fn(*concat_in, *concat_zeros)
        out_arrs = self._jax.block_until_ready(out_arrs)
        return [
            {
                name: np.asarray(out_arrs[i]).reshape(
                    NCORES, *self._out_avals[i].shape)[c]
                for i, name in enumerate(self._out_names)
            }
            for c in range(NCORES)
        ]


class _Results:
    def __init__(self, results):
        self.results = results


def _get_runner(n_reps=1, donate=True):
    key = (n_reps, donate)
    if key not in _CACHE:
        _CACHE[key] = _Runner(_build(n_reps), donate=donate)
    return _CACHE[key]


def _run(in_maps, n_reps=1):
    return _Results(_get_runner(n_reps)(in_maps))


def kernel(**inputs) -> np.ndarray:
    res = _run(_shard_inputs(inputs))
    out = np.empty((B, C, H, W, D), np.float32)
    for ci in range(NCORES):
        sl = slice(HS * ci, HS * (ci + 1))
        out[:, :, sl] = res.results[ci]["out"].reshape(B, C, HS, W, D)
    return out



# revision 50
# speedup vs baseline: 31.7581x; 31.7581x over previous
"""Trainium2 Bass kernel for nn_DecoderCrossAttention.

Reference computation (per voxel v, batch b):
    q = Wq x_v + bq                        (x = decoder_features, [C])
    k_j = Wk y_jv + bk, v_j = Wv y_jv + bv (y = skip features, COND=4 frames)
    s_j[h] = <q_h, k_jh> / sqrt(DH)        (NH=8 heads of DH=16)
    attn = softmax_j(s)                    (over the 4 conditioning frames)
    o = Wo (sum_j attn_j * v_j) + bo + x_v
    out = GroupNorm8(o) * gamma + beta     (stats over (C/G, H, W, D) per batch)

Strategy (8 NeuronCores, data-parallel over H):
  * Each core gets H-slice of 4 planes: 2*4*32*32 = 8192 voxels.
  * Feature-major layout [C=128 partitions, voxels in free dim], 512-voxel tiles.
  * x/y/Wq/Wk/Wv in bf16 (host-cast, host-transposed weights): halves input
    DMA; rel-err gate is 2e-2, measured ~2.4e-3.
  * Per-head score reduction (sum over the 16 channels of a head) and the
    softmax broadcast (8 head rows -> 128 channels) are PE matmuls against
    0/1 masks built in-kernel with iota+compare.
  * Softmax over only 4 logits, inputs are bounded => no max subtraction.
  * E~ = exp(s)*recip(Z): exp on Act, recip on DVE, the product on Pool.
  * qk = k*q via a stride-0 broadcast AP of q over the 2-cond pair (one
    PSUM operand only - NCC_IBVF027).  bk is dropped entirely: within a
    head it shifts all 4 logits by the same q.bk, which softmax cancels
    exactly.
  * attn*V products (per-cond, through two ping-ponged 1-bank PSUM pools
    so the PE broadcast hides under the previous cond's DVE multiply) feed
    4 accumulating output-projection matmuls, the residual rides in as a
    5th identity matmul, and the Act engine evacuates psO with
    bias+per-channel sums (accum_out); ssq via Act Square.  (GPSIMD
    cannot access PSUM on HW, so every PSUM evacuation stays on Act/DVE.)
  * GroupNorm is global: per-channel sum/sumsq AllGather (15us fixed vs
    28us for AllReduce in the cost model) + local reduce; a
    zero-contribution stt gates the batch-0 finalization into the batch-1
    AllGather window.  Group stats are reduced+broadcast+meaned in ONE
    pre-scaled [C,C] group-mask matmul, variance via sqrt(scale=-1,
    bias=eps), and the rescale runs on tile-PAIRS split over DVE (2x
    all-SBUF mode) / Act / Pool with paired [C, 2*NT] stores.
  * Both per-tile stats live in ONE [C, 2*B*NTILES] tile (sums | ssqs)
    so each batch's AllGather input is a single strided TensorReduce.
  * THE key multi-rep optimization: each rep's batch-1 finalization
    (gather-DMA + stats chain + rescale + stores, all gated on that
    rep's 15us AllGather) is EMITTED INTO THE NEXT REP'S instruction
    stream, after its tile 3.  The engine SEQs are in-order, so a wait
    placed at the end of rep r would block every engine's rep r+1 work
    behind the collective; deferred emission lets rep r+1's front run
    during rep r's AllGather window, cutting the marginal per-rep time
    from 134.3us to 112.3us in the TimelineSim cost model.
  * Constants are packed into 3 DMAs (wqkv, wo, vec6) and y is fetched
    per-cond with a 2-tile prefetch: first matmul fires at ~5us.

The walrus build here accepts only ONE sync wait per instruction; Tile
attaches many.  split_waits() hoists extras onto standalone EventSemaphore
instructions post-scheduling.
"""

import sys

if "/opt/trn_rl_repo" not in sys.path:
    sys.path.insert(0, "/opt/trn_rl_repo")

import numpy as np

B, COND, C, H, W, D = 2, 4, 128, 32, 32, 32
NH, DH, G = 8, 16, 8
EPS = 1e-5
NCORES = 8
HS = H // NCORES          # 4 H-planes per core
NVOX = HS * W * D         # 4096 voxels per batch per core
NT = 512                  # voxels per tile
NTILES = NVOX // NT       # 8 tiles per batch
N_GROUP = (C // G) * H * W * D   # elements per (batch, group) for GN stats

_CACHE = {}


def _split_waits(nc):
    """Hoist extra sync waits onto standalone EventSemaphore instructions."""
    from concourse import mybir
    import bass_rust

    n_split = 0
    for func in nc.m.functions:
        for blk in func.blocks:
            new_list = []
            changed = False
            for inst in blk.instructions:
                si = inst.sync_info
                waits = list(si.on_wait) if si is not None else []
                if len(waits) > 1:
                    changed = True
                    for w in waits[:-1]:
                        ev = mybir.InstEventSemaphore(
                            name=f"wsplit-{nc.next_id()}", ins=[], outs=[]
                        )
                        ev.engine = inst.engine
                        ev.sync_info = bass_rust.SyncInfo(on_wait=[w], on_update=[])
                        new_list.append(ev)
                        n_split += 1
                    inst.sync_info = bass_rust.SyncInfo(
                        on_wait=[waits[-1]], on_update=list(si.on_update)
                    )
                new_list.append(inst)
            if changed:
                blk.instructions = new_list
    return n_split


def _build(n_reps=1):
    import concourse.bass as bass
    import concourse.tile as tile
    from concourse import mybir
    from concourse.bass_isa import ReduceOp
    from contextlib import ExitStack

    dt = mybir.dt
    f32 = dt.float32
    f32r = dt.float32r
    i32 = dt.int32
    Alu = mybir.AluOpType
    Act = mybir.ActivationFunctionType
    ts = bass.ts

    bf16 = dt.bfloat16

    nc = bass.Bass("TRN2", target_bir_lowering=False, debug=False,
                   num_devices=NCORES)
    x_io = nc.dram_tensor("x", [B, C, NVOX], bf16, kind="ExternalInput").ap()
    y_io = nc.dram_tensor("y", [B, COND, C, NVOX], bf16, kind="ExternalInput").ap()
    # constants packed into 3 tensors: each dma_start costs ~0.6us of HWDGE
    # issue time, so 10 separate loads would delay the first x/y input DMAs
    wqkv_io = nc.dram_tensor("wqkv", [C, 3 * C], bf16, kind="ExternalInput").ap()
    wo_io = nc.dram_tensor("wo", [C, C], f32r, kind="ExternalInput").ap()
    vec6_io = nc.dram_tensor("vec6", [C, 6], f32, kind="ExternalInput").ap()
    out_io = nc.dram_tensor("out", [B, C, NVOX], f32, kind="ExternalOutput").ap()

    def mm(out, lhsT, rhs, start=True, stop=True):
        nc.tensor.matmul(out, lhsT=lhsT, rhs=rhs, start=start, stop=stop)

    with tile.TileContext(nc) as tc, ExitStack() as ctx:
        # ---------------- constants / weights / masks -------------------
        const = ctx.enter_context(tc.tile_pool(name="const", bufs=1))
        dram = ctx.enter_context(tc.tile_pool(name="dram", bufs=1, space="DRAM"))

        # constants first, on the Activation HWDGE queue, packed: 3 issues
        wqkv = const.tile([C, 3 * C], bf16, tag="wqkv")
        nc.scalar.dma_start(wqkv[:], wqkv_io[:])
        wo_t = const.tile([C, C], f32r, tag="wT_wo")
        nc.scalar.dma_start(wo_t[:], wo_io[:])
        vec6 = const.tile([C, 6], f32, tag="vec6")
        nc.scalar.dma_start(vec6[:], vec6_io[:])
        wT = {"wq": wqkv[:, 0:C], "wk": wqkv[:, C:2 * C],
              "wv": wqkv[:, 2 * C:3 * C], "wo": wo_t[:]}
        vecs = {name: vec6[:, i:i + 1] for i, name in
                enumerate(("bq", "bk", "bv", "bo", "gamma", "beta"))}

        # --- masks via iota + compare (int32), cast to f32
        with tc.tile_pool(name="setup", bufs=1) as setup:
            def icast(dst_ap, src_ap):
                nc.vector.tensor_copy(dst_ap, src_ap)

            # partition-index and free-index helpers
            p128 = setup.tile([C, C], i32, tag="p128")
            nc.gpsimd.iota(p128[:], pattern=[[0, C]], base=0, channel_multiplier=1)
            f128 = setup.tile([C, C], i32, tag="f128")
            nc.gpsimd.iota(f128[:], pattern=[[1, C]], base=0, channel_multiplier=0)
            hc128 = setup.tile([C, C], i32, tag="hc128")
            nc.vector.tensor_scalar(hc128[:], p128[:], 4, None,
                                    Alu.arith_shift_right)
            tmpi = setup.tile([C, C], i32, tag="tmpi")

            # identity [128,128] (bf16, for the residual pass-through matmul)
            ident = const.tile([C, C], bf16, tag="ident")
            nc.vector.tensor_tensor(tmpi[:], f128[:], p128[:], Alu.is_equal)
            icast(ident[:], tmpi[:])

            # mask32 [128, 4*32]: col 32j+m ; 1 iff (m - 8j) == c//16
            jm = setup.tile([C, C], i32, tag="jm")
            nc.gpsimd.iota(jm[:].rearrange("p (j m) -> p j m", j=4),
                           pattern=[[-8, 4], [1, 32]], base=0,
                           channel_multiplier=0)
            mask32 = const.tile([C, C], bf16, tag="mask32")
            nc.vector.tensor_tensor(tmpi[:], jm[:], hc128[:], Alu.is_equal)
            icast(mask32[:], tmpi[:])

            # lhsT32 [32,32]: 1 iff p%8 == m%8  (Z replication matmul)
            p32 = setup.tile([32, 32], i32, tag="p32")
            nc.gpsimd.iota(p32[:], pattern=[[0, 32]], base=0, channel_multiplier=1)
            pm32 = setup.tile([32, 32], i32, tag="pm32")
            nc.vector.tensor_scalar(pm32[:], p32[:], 3, 3,
                                    Alu.arith_shift_right, Alu.arith_shift_left)
            t32 = setup.tile([32, 32], i32, tag="t32")
            nc.vector.tensor_tensor(t32[:], p32[:], pm32[:], Alu.subtract)
            fm32 = setup.tile([32, 32], i32, tag="fm32")
            nc.gpsimd.iota(fm32[:].rearrange("p (j m) -> p j m", j=4),
                           pattern=[[0, 4], [1, 8]], base=0, channel_multiplier=0)
            e32 = setup.tile([32, 32], i32, tag="e32")
            nc.vector.tensor_tensor(e32[:], fm32[:], t32[:], Alu.is_equal)
            lhsT32 = const.tile([32, 32], f32r, tag="lhsT32")
            icast(lhsT32[:], e32[:])

            # maskb [32, 4*128]: col 128j+c ; 1 iff (p - 8j) == c//16
            pj = setup.tile([32, 4 * C], i32, tag="pj")
            nc.gpsimd.iota(pj[:].rearrange("p (j c) -> p j c", j=4),
                           pattern=[[-8, 4], [0, C]], base=0,
                           channel_multiplier=1)
            fc = setup.tile([32, 4 * C], i32, tag="fc")
            nc.gpsimd.iota(fc[:].rearrange("p (j c) -> p j c", j=4),
                           pattern=[[0, 4], [1, C]], base=0, channel_multiplier=0)
            nc.vector.tensor_scalar(fc[:], fc[:], 4, None, Alu.arith_shift_right)
            eb = setup.tile([32, 4 * C], i32, tag="eb")
            nc.vector.tensor_tensor(eb[:], pj[:], fc[:], Alu.is_equal)
            maskb = const.tile([32, 4 * C], bf16, tag="maskb")
            icast(maskb[:], eb[:])

            # ggmask [128, 128]: 1/N_GROUP iff p//16 == c//16  (GN group sum,
            # fused reduce+broadcast+mean: psGB = ggmask.T @ stats gives the
            # group means [mean, E[x^2]] directly at channel layout)
            fg = setup.tile([C, C], i32, tag="fg")
            nc.vector.tensor_scalar(fg[:], f128[:], 4, None,
                                    Alu.arith_shift_right)
            egg = setup.tile([C, C], i32, tag="egg")
            nc.vector.tensor_tensor(egg[:], fg[:], hc128[:], Alu.is_equal)
            ggmask = const.tile([C, C], f32, tag="ggmask")
            icast(ggmask[:], egg[:])
            nc.vector.tensor_scalar(ggmask[:], ggmask[:], 1.0 / N_GROUP, None,
                                    Alu.mult)
            eps_t = const.tile([C, 1], f32, tag="eps_t")
            nc.vector.memset(eps_t[:], EPS)

        # ---------------- main pipeline ---------------------------------
        per_rep_pools = dict(
            xres=ctx.enter_context(tc.tile_pool(name="xres", bufs=2)),
            ypool=ctx.enter_context(tc.tile_pool(name="ypool", bufs=2)),
            sb=ctx.enter_context(tc.tile_pool(name="sb", bufs=2)),
            big=ctx.enter_context(tc.tile_pool(name="bigsb", bufs=2)),
            # bufs=2 so rep r+1's accumulation/stats don't serialize behind
            # rep r's finalization (cross-rep overlap; no-op for n_reps=1)
            opool=ctx.enter_context(tc.tile_pool(name="opool", bufs=2)),
            stats=ctx.enter_context(tc.tile_pool(name="stats", bufs=2)),
            ps_kb=ctx.enter_context(tc.tile_pool(name="ps_kb", bufs=1, space="PSUM")),
            ps_bb1=ctx.enter_context(tc.tile_pool(name="ps_bb1", bufs=1, space="PSUM")),
            ps_bb2=ctx.enter_context(tc.tile_pool(name="ps_bb2", bufs=1, space="PSUM")),
            ps_q=ctx.enter_context(tc.tile_pool(name="ps_q", bufs=1, space="PSUM")),
            ps_v=ctx.enter_context(tc.tile_pool(name="ps_v", bufs=1, space="PSUM")),
            ps_s=ctx.enter_context(tc.tile_pool(name="ps_s", bufs=1, space="PSUM")),
            ps_o=ctx.enter_context(tc.tile_pool(name="ps_o", bufs=1, space="PSUM")),
        )

        def emit_rep(rep, deferred):
            """Emit one rep; returns this rep's deferred b1 finalization.

            deferred: previous rep's b1 finalization, emitted into THIS
            rep's stream after tile 3 so its AllGather wait is already
            satisfied when each engine SEQ reaches it.  A nested function
            so each rep's closures bind their own tiles (the rep loop
            would otherwise rebind shared locals under the deferred call).
            """
            p = per_rep_pools
            out_acc = p["opool"].tile([C, B * NVOX], f32, tag="out_acc")
            # per-tile stats, one tile: col b*NTILES+t = sums, col
            # B*NTILES + b*NTILES + t = sums-of-squares (so each batch's
            # sums and ssqs reduce in ONE strided TensorReduce)
            BN = B * NTILES
            st2 = p["stats"].tile([C, 2 * BN], f32, tag="st2")
            dump = p["stats"].tile([C, NT], f32, tag="dump")

            tiles = [(b, t) for b in range(B) for t in range(NTILES)]
            xres_b = {}
            ychunk_state = {}
            fstate = {}
            sstate = {}

            def load_x(b):
                if b in xres_b:
                    return
                xr = p["xres"].tile([C, NVOX], bf16, tag="xres")
                nc.sync.dma_start(xr[:], x_io[b])
                xres_b[b] = xr

            def load_ychunk(b, ci):
                if (b, ci) in ychunk_state:
                    return
                yc = p["ypool"].tile([C, COND * 4 * NT], bf16, tag="ychunk")
                ysrc = y_io[b].rearrange("j c v -> c j v")
                # per-cond DMAs: K-proj for cond j only waits on its slice
                for j in range(COND):
                    nc.sync.dma_start(
                        yc[:, j * 4 * NT: (j + 1) * 4 * NT],
                        ysrc[:, bass.ts(j, 1), bass.ts(ci, 4 * NT)]
                        .rearrange("p j v -> p (j v)"),
                    )
                ychunk_state[(b, ci)] = yc

            def front1(k):
                """DMAs, Q proj, K h0 proj, V proj+copies."""
                b, t = tiles[k]
                load_x(b)
                load_ychunk(b, t // 4)
                ychunk = ychunk_state[(b, t // 4)]
                yj_of = lambda j: ychunk[:, j * 4 * NT + (t % 4) * NT:
                                         j * 4 * NT + (t % 4 + 1) * NT]
                xt = xres_b[b][:, ts(t, NT)]
                psQ = p["ps_q"].tile([C, NT], f32, tag="psq")
                mm(psQ[:], wT["wq"], xt)
                qsb = p["sb"].tile([C, NT], bf16, tag="qsb")
                nc.scalar.activation(qsb[:], psQ[:], Act.Identity,
                                     bias=vecs["bq"])
                psKB0 = p["ps_kb"].tile([C, 2 * NT], f32, tag="kb")
                for j in (0, 1):
                    mm(psKB0[:, ts(j, NT)], wT["wk"], yj_of(j))
                vbig = p["big"].tile([C, COND * NT], f32, tag="vbig")
                for j in range(COND):
                    psV = p["ps_v"].tile([C, NT], f32, tag="psv")
                    mm(psV[:], wT["wv"], yj_of(j))
                    nc.scalar.activation(vbig[:, ts(j, NT)], psV[:],
                                         Act.Identity, bias=vecs["bv"])
                qkbig = p["big"].tile([C, COND * NT], bf16, tag="qkbig")
                psS = p["ps_s"].tile([32, NT], f32, tag="pss")
                fstate[k] = (psS, vbig, xt, qsb, qkbig, yj_of, psKB0)
                # prefetch the next y chunk / next batch's inputs during the
                # current chunk's idle DMA time (avoids a burst at t%4==0 and
                # at the batch boundary, where stores also compete)
                if t % 4 == 2:
                    if t // 4 + 1 < NTILES // 4:
                        load_ychunk(b, t // 4 + 1)
                    elif b + 1 < B:
                        load_x(b + 1)
                        load_ychunk(b + 1, 0)

            def front_qk(k, h):
                """QK mul half h + its score matmuls (+ K h1 projections).

                qk = k * q via a stride-0 broadcast of qsb over the 2-cond
                pair (one PSUM operand only: NCC_IBVF027).  bk is dropped
                entirely: within a head it shifts all 4 logits by the same
                q.bk, which softmax cancels exactly.
                """
                psS, vbig, xt, qsb, qkbig, yj_of, psKB = fstate[k]
                qrep = qsb[:].unsqueeze(1).broadcast_to([C, 2, NT])
                nc.vector.tensor_tensor(
                    qkbig[:, ts(h, 2 * NT)].rearrange("p (j v) -> p j v", j=2),
                    psKB[:].rearrange("p (j v) -> p j v", j=2),
                    qrep, Alu.mult)
                for j in (2 * h, 2 * h + 1):
                    mm(psS[:], mask32[:, ts(j, 32)], qkbig[:, ts(j, NT)],
                       start=(j == 0), stop=(j == COND - 1))
                if h == 0:
                    psKB1 = p["ps_kb"].tile([C, 2 * NT], f32, tag="kb")
                    for j in (2, 3):
                        mm(psKB1[:, ts(j - 2, NT)], wT["wk"], yj_of(j))
                    fstate[k] = (psS, vbig, xt, qsb, qkbig, yj_of, psKB1)

            def soft(k):
                """exp, Z-matmul, reciprocal, E~ = E * (1/Z)."""
                b, t = tiles[k]
                psS, vbig, xt, qsb, qkbig, yj_of, _ = fstate.pop(k)
                esb = p["sb"].tile([32, NT], f32r, tag="esb")
                nc.scalar.activation(esb[:], psS[:], Act.Exp, scale=0.25)
                # psZ lives in the ps_s pool: its WAR (next tile's scores)
                # sits later in the chain than ps_q's (next tile's Q-proj).
                psZ = p["ps_s"].tile([32, NT], f32, tag="pss")
                mm(psZ[:], lhsT32[:], esb[:])
                rsb = p["sb"].tile([32, NT], f32, tag="rsb")
                nc.vector.reciprocal(rsb[:], psZ[:])
                etsb = p["sb"].tile([32, NT], bf16, tag="etsb")
                nc.gpsimd.tensor_tensor(etsb[:], esb[:].bitcast(f32),
                                        rsb[:], Alu.mult)
                sstate[k] = (etsb, vbig, xt)

            def back_cond(k, j, wbig):
                """Broadcast matmul + attn*V multiply for one cond.

                Per-cond with two ping-ponged 1-bank PSUM pools: bcast j+2
                only waits on avm j, so the PE bcast hides under the DVE
                multiply of the previous cond."""
                etsb, vbig, xt = sstate[k]
                pool = p["ps_bb1"] if j % 2 == 0 else p["ps_bb2"]
                psB = pool.tile([C, NT], f32, tag="bb")
                mm(psB[:], maskb[:, ts(j, C)], etsb[:])
                nc.vector.tensor_tensor(
                    wbig[:, ts(j, NT)], psB[:],
                    vbig[:, ts(j, NT)], Alu.mult)

            def back_out(k, wbig):
                b, t = tiles[k]
                col = b * NTILES + t
                etsb, vbig, xt = sstate.pop(k)
                psO = p["ps_o"].tile([C, NT], f32, tag="pso")
                for j in range(COND):
                    mm(psO[:], wT["wo"], wbig[:, ts(j, NT)],
                       start=(j == 0), stop=False)
                # residual folded into the accumulation group: psO += I @ x
                mm(psO[:], ident[:], xt, start=False, stop=True)
                outt = out_acc[:, col * NT: (col + 1) * NT]
                nc.scalar.activation(
                    outt, psO[:], Act.Identity, bias=vecs["bo"],
                    accum_out=st2[:, col: col + 1])
                nc.scalar.activation(
                    dump[:], outt, Act.Square,
                    accum_out=st2[:, BN + col: BN + col + 1])

            cc_state = {}
            cc_sb = {}

            def gn_pre(b):
                """Reduce per-channel stats and launch the AllGather."""
                ccsb = p["stats"].tile([C, 2], f32, tag=f"ccsb{b}")
                # ONE strided reduce over [C, (2 stats, NTILES)] -> [C, 2]
                nc.vector.reduce_sum(
                    ccsb[:],
                    st2[:].rearrange("p (s x) -> p s x", s=2)
                    [:, :, b * NTILES:(b + 1) * NTILES],
                    axis=mybir.AxisListType.X)
                cc_in = dram.tile([C, 2], f32, tag=f"cc_in{b}")
                cc_out = dram.tile([NCORES, C, 2], f32, tag=f"cc_out{b}")
                # Act HWDGE queue: must not sit behind bulk stores on SP
                nc.scalar.dma_start(cc_in[:], ccsb[:])
                nc.gpsimd.collective_compute(
                    "AllGather", Alu.bypass,
                    replica_groups=[list(range(NCORES))],
                    ins=[cc_in.opt()], outs=[cc_out.opt()])
                cc_state[b] = cc_out
                cc_sb[b] = ccsb

            def gn_post(b):
                """Stats -> per-channel affine -> rescale out_acc -> store."""
                cc_out = cc_state.pop(b)
                # gather the 8 cores' [C,2] partials: gs16[:, 0:8]=sums,
                # gs16[:, 8:16]=ssqs (s-major so the reduces are contiguous)
                gs16 = p["stats"].tile([C, 16], f32, tag=f"gs16_{b}")
                nc.sync.dma_start(
                    gs16[:].rearrange("p (s n) -> p s n", n=NCORES),
                    cc_out[:].rearrange("n p s -> p s n"))
                # one reduce over cores -> per-channel sums [C, 2]
                gsb = p["stats"].tile([C, 2], f32, tag=f"gsb{b}")
                nc.vector.reduce_sum(
                    gsb[:], gs16[:].rearrange("p (s n) -> p s n", n=NCORES),
                    axis=mybir.AxisListType.X)
                # fused group reduce+broadcast+mean: psGB = [mean, E[x^2]].
                # The ps_q WAR this creates is harmless: for b=1 this op is
                # emitted after the NEXT rep's tile 3 (deferred), so the
                # next rep's early Q-projections precede it in the stream.
                psGB = p["ps_q"].tile([C, 2], f32, tag="psq")
                nc.tensor.matmul(psGB[:], lhsT=ggmask[:], rhs=gsb[:],
                                 start=True, stop=True)
                msb = p["stats"].tile([C, 2], f32, tag=f"msb{b}")
                nc.vector.tensor_copy(msb[:], psGB[:])
                # negvar = mean^2 - E[x^2]; rstd = 1/sqrt(eps - negvar)
                nvar = p["stats"].tile([C, 1], f32, tag=f"nvar{b}")
                nc.vector.scalar_tensor_tensor(
                    nvar[:], msb[:, 0:1], msb[:, 0:1], msb[:, 1:2],
                    Alu.mult, Alu.subtract)
                sstd = p["stats"].tile([C, 1], f32, tag=f"sstd{b}")
                nc.scalar.activation(sstd[:], nvar[:], Act.Sqrt,
                                     bias=eps_t[:], scale=-1.0)
                rstd = p["stats"].tile([C, 1], f32, tag=f"rstd{b}")
                nc.vector.reciprocal(rstd[:], sstd[:])
                scale_b = p["stats"].tile([C, 1], f32, tag=f"scale{b}")
                nc.vector.tensor_tensor(scale_b[:], rstd[:],
                                        vecs["gamma"], Alu.mult)
                gate = cc_sb.get(1 - b) if b == 0 else None
                if gate is not None:
                    # 0*ccsb(b1) + scale: a zero-contribution data dependency
                    # that stops the scheduler hoisting b0's rescale+stores
                    # out of the b1 AllGather window they are meant to fill
                    gated = p["stats"].tile([C, 1], f32, tag=f"gsc{b}")
                    nc.vector.scalar_tensor_tensor(
                        gated[:], gate[:, 0:1], 0.0, scale_b[:],
                        Alu.mult, Alu.add)
                    scale_b = gated
                negb_b = p["stats"].tile([C, 2], f32, tag=f"negb{b}")
                nc.vector.scalar_tensor_tensor(
                    negb_b[:, 0:1], msb[:, 0:1], scale_b[:],
                    vecs["beta"], Alu.mult, Alu.subtract)
                nc.vector.tensor_scalar(negb_b[:, 1:2], negb_b[:, 0:1],
                                        -1.0, None, Alu.mult)
                # rescale tile-PAIRS split over three engines (DVE 2x all-
                # SBUF mode / Act / Pool), storing each pair once rescaled
                for tp in range(NTILES // 2):
                    src = out_acc[:, (b * NTILES + 2 * tp) * NT:
                                  (b * NTILES + 2 * tp + 2) * NT]
                    if tp == 3:
                        nc.gpsimd.tensor_scalar(
                            src, src,
                            scale_b[:], negb_b[:, 0:1], Alu.mult, Alu.subtract)
                    elif tp == 1:
                        nc.scalar.activation(src, src, Act.Identity,
                                             scale=scale_b[:],
                                             bias=negb_b[:, 1:2])
                    else:
                        nc.vector.tensor_scalar(
                            src, src,
                            scale_b[:], negb_b[:, 0:1], Alu.mult, Alu.subtract)
                    nc.sync.dma_start(out_io[b][:, ts(tp, 2 * NT)], src)

            NK = len(tiles)
            for k in range(NK + 2):
                if k == 4 and deferred is not None:
                    deferred()
                    deferred = None
                if 1 <= k <= NK:
                    soft(k - 1)
                if k >= 2:
                    wbig = p["big"].tile([C, COND * NT], f32r, tag="qkbig")
                    back_cond(k - 2, 0, wbig)
                    back_cond(k - 2, 1, wbig)
                if k < NK:
                    front1(k)
                    front_qk(k, 0)
                if k >= 2:
                    back_cond(k - 2, 2, wbig)
                    back_cond(k - 2, 3, wbig)
                if k < NK:
                    front_qk(k, 1)
                if k >= 2:
                    back_out(k - 2, wbig)
                    bdone, tdone = tiles[k - 2]
                    if tdone == NTILES - 1:
                        gn_pre(bdone)
            # gn_post(0) emits now: it depends only on the (long-finished)
            # b0 AllGather and its gated rescale+stores fill the b1
            # AllGather latency.  gn_post(1) is deferred into the NEXT
            # rep's stream (emitted after its tile 3) so the b1 AllGather
            # wait never blocks an engine SEQ ahead of next-rep work.
            gn_post(0)
            return lambda: gn_post(1)

        pending = None
        for rep in range(n_reps):
            pending = emit_rep(rep, pending)
        pending()


    _split_waits(nc)
    return nc


def _shard_inputs(inputs):
    import ml_dtypes
    bf16 = ml_dtypes.bfloat16
    x = np.asarray(inputs["decoder_features"], np.float32).astype(bf16)
    y = np.asarray(inputs["skip_connection_features"], np.float32).astype(bf16)

    def wT(name, dtype):
        w = np.asarray(inputs[name], np.float32)
        return np.ascontiguousarray(w.T).astype(dtype)

    wqkv = np.concatenate([wT("w_q", bf16), wT("w_k", bf16),
                           wT("w_v", bf16)], axis=1)
    vec6 = np.stack([np.asarray(inputs[n], np.float32) for n in
                     ("b_q", "b_k", "b_v", "b_o", "gn_gamma", "gn_beta")],
                    axis=1)
    base = {
        "wqkv": np.ascontiguousarray(wqkv),
        "wo": wT("w_o", np.float32),
        "vec6": np.ascontiguousarray(vec6),
    }
    in_maps = []
    for ci in range(NCORES):
        sl = slice(HS * ci, HS * (ci + 1))
        im = dict(base)
        im["x"] = np.ascontiguousarray(x[:, :, sl]).reshape(B, C, NVOX)
        im["y"] = np.ascontiguousarray(y[:, :, :, sl]).reshape(B, COND, C, NVOX)
        in_maps.append(im)
    return in_maps


class _Runner:
    """Persistent PJRT runner: trace/compile once, execute many times.

    Mirrors concourse.bass2jax.run_bass_via_pjrt's multi-core branch but
    keeps the jitted shard_map callable alive so repeat calls skip
    re-tracing and NEFF recompilation.
    """

    def __init__(self, nc, donate=True):
        import jax
        from jax.sharding import Mesh, PartitionSpec
        from jax.experimental.shard_map import shard_map
        from concourse import bass2jax, mybir

        bass2jax.install_neuronx_cc_hook()
        assert nc.dbg_addr is None
        partition_name = (nc.partition_id_tensor.name
                          if nc.partition_id_tensor else None)
        in_names, out_names, out_avals, zero_outs = [], [], [], []
        for alloc in nc.m.functions[0].allocations:
            if not isinstance(alloc, mybir.MemoryLocationSet):
                continue
            name = alloc.memorylocations[0].name
            if alloc.kind == "ExternalInput":
                if name != partition_name:
                    in_names.append(name)
            elif alloc.kind == "ExternalOutput":
                out_names.append(name)
                shape = tuple(alloc.tensor_shape)
                dtype = mybir.dt.np(alloc.dtype)
                out_avals.append(jax.core.ShapedArray(shape, dtype))
                zero_outs.append(np.zeros(shape, dtype))
        n_params = len(in_names)
        n_outs = len(out_avals)
        in_names.extend(out_names)
        if partition_name is not None:
            in_names.append(partition_name)
        donate_idx = tuple(range(n_params, n_params + n_outs)) if donate else ()

        def _body(*args):
            operands = list(args)
            if partition_name is not None:
                operands.append(bass2jax.partition_id_tensor())
            outs = bass2jax._bass_exec_p.bind(
                *operands,
                out_avals=tuple(out_avals),
                in_names=tuple(in_names),
                out_names=tuple(out_names),
                lowering_input_output_aliases=(),
                sim_require_finite=True,
                sim_require_nnan=True,
                nc=nc,
            )
            return tuple(outs)

        devices = jax.devices()[:NCORES]
        mesh = Mesh(np.asarray(devices), ("core",))
        in_specs = (PartitionSpec("core"),) * (n_params + n_outs)
        out_specs = (PartitionSpec("core"),) * n_outs
        self._fn = jax.jit(
            shard_map(_body, mesh=mesh, in_specs=in_specs,
                      out_specs=out_specs, check_rep=False),
            donate_argnums=donate_idx, keep_unused=True)
        self._in_names = in_names[:n_params]
        self._out_names = out_names
        self._out_avals = out_avals
        self._zero_outs = zero_outs
        self._jax = jax

    def __call__(self, in_maps):
        concat_in = [
            np.concatenate([np.asarray(m[name]) for m in in_maps], axis=0)
            for name in self._in_names
        ]
        concat_zeros = [
            np.zeros((NCORES * z.shape[0], *z.shape[1:]), z.dtype)
            for z in self._zero_outs
        ]
        out_arrs = self._fn(*concat_in, *concat_zeros)
        out_arrs = self._jax.block_until_ready(out_arrs)
        return [
            {
                name: np.asarray(out_arrs[i]).reshape(
                    NCORES, *self._out_avals[i].shape)[c]
                for i, name in enumerate(self._out_names)
            }
            for c in range(NCORES)
        ]


class _Results:
    def __init__(self, results):
        self.results = results


def _get_runner(n_reps=1, donate=True):
    key = (n_reps, donate)
    if key not in _CACHE:
        _CACHE[key] = _Runner(_build(n_reps), donate=donate)
    return _CACHE[key]


def _run(in_maps, n_reps=1):
    return _Results(_get_runner(n_reps)(in_maps))


def kernel(**inputs) -> np.ndarray:
    res = _run(_shard_inputs(inputs))
    out = np.empty((B, C, H, W, D), np.float32)
    for ci in range(NCORES):
        sl = slice(HS * ci, HS * (ci + 1))
        out[:, :, sl] = res.results[ci]["out"].reshape(B, C, HS, W, D)
    return out



# revision 55
# speedup vs baseline: 32.1249x; 1.0115x over previous
"""Trainium2 Bass kernel for nn_DecoderCrossAttention.

Reference computation (per voxel v, batch b):
    q = Wq x_v + bq                        (x = decoder_features, [C])
    k_j = Wk y_jv + bk, v_j = Wv y_jv + bv (y = skip features, COND=4 frames)
    s_j[h] = <q_h, k_jh> / sqrt(DH)        (NH=8 heads of DH=16)
    attn = softmax_j(s)                    (over the 4 conditioning frames)
    o = Wo (sum_j attn_j * v_j) + bo + x_v
    out = GroupNorm8(o) * gamma + beta     (stats over (C/G, H, W, D) per batch)

Strategy (8 NeuronCores, data-parallel over H):
  * Each core gets H-slice of 4 planes: 2*4*32*32 = 8192 voxels.
  * Feature-major layout [C=128 partitions, voxels in free dim], 512-voxel tiles.
  * x/y/Wq/Wk/Wv in bf16 (host-cast, host-transposed weights): halves input
    DMA; rel-err gate is 2e-2, measured ~2.4e-3.
  * Per-head score reduction (sum over the 16 channels of a head) and the
    softmax broadcast (8 head rows -> 128 channels) are PE matmuls against
    0/1 masks built in-kernel with iota+compare.
  * Softmax over only 4 logits, inputs are bounded => no max subtraction.
  * E~ = exp(s)*recip(Z): exp on Act, recip on DVE, the product on Pool.
  * qk = k*q via a stride-0 broadcast AP of q over the 2-cond pair (one
    PSUM operand only - NCC_IBVF027).  bk is dropped entirely: within a
    head it shifts all 4 logits by the same q.bk, which softmax cancels
    exactly.
  * attn*V products (per-cond, through two ping-ponged 1-bank PSUM pools
    so the PE broadcast hides under the previous cond's DVE multiply) feed
    4 accumulating output-projection matmuls, the residual rides in as a
    5th identity matmul, and the Act engine evacuates psO with
    bias+per-channel sums (accum_out); ssq via Act Square.  (GPSIMD
    cannot access PSUM on HW, so every PSUM evacuation stays on Act/DVE.)
  * GroupNorm is global: per-channel sum/sumsq AllGather (15us fixed vs
    28us for AllReduce in the cost model) + local reduce; a
    zero-contribution stt gates the batch-0 finalization into the batch-1
    AllGather window.  Group stats are reduced+broadcast+meaned in ONE
    pre-scaled [C,C] group-mask matmul, variance via sqrt(scale=-1,
    bias=eps), and the rescale runs on tile-PAIRS split DVE/Act/Pool/
    Pool (Pool idles at the rep boundary) with paired [C, 2*NT] stores.
  * Both per-tile stats live in ONE [C, 2*B*NTILES] tile (sums | ssqs)
    so each batch's AllGather input is a single strided TensorReduce.
  * THE key multi-rep optimization: each rep's batch-1 finalization
    (gather-DMA + stats chain + rescale + stores, all gated on that
    rep's 15us AllGather) is EMITTED INTO THE NEXT REP'S instruction
    stream, after its tile 5.  The engine SEQs are in-order, so a wait
    placed at the end of rep r would block every engine's rep r+1 work
    behind the collective; deferred emission lets rep r+1's front run
    during rep r's AllGather window, cutting the marginal per-rep time
    from 134.3us to 111.0us in the TimelineSim cost model.
  * Constants are packed into 3 DMAs (wqkv, wo, vec6) and y is fetched
    per-cond with a 2-tile prefetch: first matmul fires at ~5us.

The walrus build here accepts only ONE sync wait per instruction; Tile
attaches many.  split_waits() hoists extras onto standalone EventSemaphore
instructions post-scheduling.
"""

import sys

if "/opt/trn_rl_repo" not in sys.path:
    sys.path.insert(0, "/opt/trn_rl_repo")

import numpy as np

B, COND, C, H, W, D = 2, 4, 128, 32, 32, 32
NH, DH, G = 8, 16, 8
EPS = 1e-5
NCORES = 8
HS = H // NCORES          # 4 H-planes per core
NVOX = HS * W * D         # 4096 voxels per batch per core
NT = 512                  # voxels per tile
NTILES = NVOX // NT       # 8 tiles per batch
N_GROUP = (C // G) * H * W * D   # elements per (batch, group) for GN stats

_CACHE = {}


def _split_waits(nc):
    """Hoist extra sync waits onto standalone EventSemaphore instructions."""
    from concourse import mybir
    import bass_rust

    n_split = 0
    for func in nc.m.functions:
        for blk in func.blocks:
            new_list = []
            changed = False
            for inst in blk.instructions:
                si = inst.sync_info
                waits = list(si.on_wait) if si is not None else []
                if len(waits) > 1:
                    changed = True
                    for w in waits[:-1]:
                        ev = mybir.InstEventSemaphore(
                            name=f"wsplit-{nc.next_id()}", ins=[], outs=[]
                        )
                        ev.engine = inst.engine
                        ev.sync_info = bass_rust.SyncInfo(on_wait=[w], on_update=[])
                        new_list.append(ev)
                        n_split += 1
                    inst.sync_info = bass_rust.SyncInfo(
                        on_wait=[waits[-1]], on_update=list(si.on_update)
                    )
                new_list.append(inst)
            if changed:
                blk.instructions = new_list
    return n_split


def _build(n_reps=1):
    import concourse.bass as bass
    import concourse.tile as tile
    from concourse import mybir
    from concourse.bass_isa import ReduceOp
    from contextlib import ExitStack

    dt = mybir.dt
    f32 = dt.float32
    f32r = dt.float32r
    i32 = dt.int32
    Alu = mybir.AluOpType
    Act = mybir.ActivationFunctionType
    ts = bass.ts

    bf16 = dt.bfloat16

    nc = bass.Bass("TRN2", target_bir_lowering=False, debug=False,
                   num_devices=NCORES)
    x_io = nc.dram_tensor("x", [B, C, NVOX], bf16, kind="ExternalInput").ap()
    y_io = nc.dram_tensor("y", [B, COND, C, NVOX], bf16, kind="ExternalInput").ap()
    # constants packed into 3 tensors: each dma_start costs ~0.6us of HWDGE
    # issue time, so 10 separate loads would delay the first x/y input DMAs
    wqkv_io = nc.dram_tensor("wqkv", [C, 3 * C], bf16, kind="ExternalInput").ap()
    wo_io = nc.dram_tensor("wo", [C, C], f32r, kind="ExternalInput").ap()
    vec6_io = nc.dram_tensor("vec6", [C, 6], f32, kind="ExternalInput").ap()
    out_io = nc.dram_tensor("out", [B, C, NVOX], f32, kind="ExternalOutput").ap()

    def mm(out, lhsT, rhs, start=True, stop=True):
        nc.tensor.matmul(out, lhsT=lhsT, rhs=rhs, start=start, stop=stop)

    with tile.TileContext(nc) as tc, ExitStack() as ctx:
        # ---------------- constants / weights / masks -------------------
        const = ctx.enter_context(tc.tile_pool(name="const", bufs=1))
        dram = ctx.enter_context(tc.tile_pool(name="dram", bufs=1, space="DRAM"))

        # constants first, on the Activation HWDGE queue, packed: 3 issues
        wqkv = const.tile([C, 3 * C], bf16, tag="wqkv")
        nc.scalar.dma_start(wqkv[:], wqkv_io[:])
        wo_t = const.tile([C, C], f32r, tag="wT_wo")
        nc.scalar.dma_start(wo_t[:], wo_io[:])
        vec6 = const.tile([C, 6], f32, tag="vec6")
        nc.scalar.dma_start(vec6[:], vec6_io[:])
        wT = {"wq": wqkv[:, 0:C], "wk": wqkv[:, C:2 * C],
              "wv": wqkv[:, 2 * C:3 * C], "wo": wo_t[:]}
        vecs = {name: vec6[:, i:i + 1] for i, name in
                enumerate(("bq", "bk", "bv", "bo", "gamma", "beta"))}

        # --- masks via iota + compare (int32), cast to f32
        with tc.tile_pool(name="setup", bufs=1) as setup:
            def icast(dst_ap, src_ap):
                nc.vector.tensor_copy(dst_ap, src_ap)

            # partition-index and free-index helpers
            p128 = setup.tile([C, C], i32, tag="p128")
            nc.gpsimd.iota(p128[:], pattern=[[0, C]], base=0, channel_multiplier=1)
            f128 = setup.tile([C, C], i32, tag="f128")
            nc.gpsimd.iota(f128[:], pattern=[[1, C]], base=0, channel_multiplier=0)
            hc128 = setup.tile([C, C], i32, tag="hc128")
            nc.vector.tensor_scalar(hc128[:], p128[:], 4, None,
                                    Alu.arith_shift_right)
            tmpi = setup.tile([C, C], i32, tag="tmpi")

            # identity [128,128] (bf16, for the residual pass-through matmul)
            ident = const.tile([C, C], bf16, tag="ident")
            nc.vector.tensor_tensor(tmpi[:], f128[:], p128[:], Alu.is_equal)
            icast(ident[:], tmpi[:])

            # mask32 [128, 4*32]: col 32j+m ; 1 iff (m - 8j) == c//16
            jm = setup.tile([C, C], i32, tag="jm")
            nc.gpsimd.iota(jm[:].rearrange("p (j m) -> p j m", j=4),
                           pattern=[[-8, 4], [1, 32]], base=0,
                           channel_multiplier=0)
            mask32 = const.tile([C, C], bf16, tag="mask32")
            nc.vector.tensor_tensor(tmpi[:], jm[:], hc128[:], Alu.is_equal)
            icast(mask32[:], tmpi[:])

            # lhsT32 [32,32]: 1 iff p%8 == m%8  (Z replication matmul)
            p32 = setup.tile([32, 32], i32, tag="p32")
            nc.gpsimd.iota(p32[:], pattern=[[0, 32]], base=0, channel_multiplier=1)
            pm32 = setup.tile([32, 32], i32, tag="pm32")
            nc.vector.tensor_scalar(pm32[:], p32[:], 3, 3,
                                    Alu.arith_shift_right, Alu.arith_shift_left)
            t32 = setup.tile([32, 32], i32, tag="t32")
            nc.vector.tensor_tensor(t32[:], p32[:], pm32[:], Alu.subtract)
            fm32 = setup.tile([32, 32], i32, tag="fm32")
            nc.gpsimd.iota(fm32[:].rearrange("p (j m) -> p j m", j=4),
                           pattern=[[0, 4], [1, 8]], base=0, channel_multiplier=0)
            e32 = setup.tile([32, 32], i32, tag="e32")
            nc.vector.tensor_tensor(e32[:], fm32[:], t32[:], Alu.is_equal)
            lhsT32 = const.tile([32, 32], f32r, tag="lhsT32")
            icast(lhsT32[:], e32[:])

            # maskb [32, 4*128]: col 128j+c ; 1 iff (p - 8j) == c//16
            pj = setup.tile([32, 4 * C], i32, tag="pj")
            nc.gpsimd.iota(pj[:].rearrange("p (j c) -> p j c", j=4),
                           pattern=[[-8, 4], [0, C]], base=0,
                           channel_multiplier=1)
            fc = setup.tile([32, 4 * C], i32, tag="fc")
            nc.gpsimd.iota(fc[:].rearrange("p (j c) -> p j c", j=4),
                           pattern=[[0, 4], [1, C]], base=0, channel_multiplier=0)
            nc.vector.tensor_scalar(fc[:], fc[:], 4, None, Alu.arith_shift_right)
            eb = setup.tile([32, 4 * C], i32, tag="eb")
            nc.vector.tensor_tensor(eb[:], pj[:], fc[:], Alu.is_equal)
            maskb = const.tile([32, 4 * C], bf16, tag="maskb")
            icast(maskb[:], eb[:])

            # ggmask [128, 128]: 1/N_GROUP iff p//16 == c//16  (GN group sum,
            # fused reduce+broadcast+mean: psGB = ggmask.T @ stats gives the
            # group means [mean, E[x^2]] directly at channel layout)
            fg = setup.tile([C, C], i32, tag="fg")
            nc.vector.tensor_scalar(fg[:], f128[:], 4, None,
                                    Alu.arith_shift_right)
            egg = setup.tile([C, C], i32, tag="egg")
            nc.vector.tensor_tensor(egg[:], fg[:], hc128[:], Alu.is_equal)
            ggmask = const.tile([C, C], f32, tag="ggmask")
            icast(ggmask[:], egg[:])
            nc.vector.tensor_scalar(ggmask[:], ggmask[:], 1.0 / N_GROUP, None,
                                    Alu.mult)
            eps_t = const.tile([C, 1], f32, tag="eps_t")
            nc.vector.memset(eps_t[:], EPS)

        # ---------------- main pipeline ---------------------------------
        per_rep_pools = dict(
            xres=ctx.enter_context(tc.tile_pool(name="xres", bufs=2)),
            ypool=ctx.enter_context(tc.tile_pool(name="ypool", bufs=2)),
            sb=ctx.enter_context(tc.tile_pool(name="sb", bufs=2)),
            big=ctx.enter_context(tc.tile_pool(name="bigsb", bufs=2)),
            # bufs=2 so rep r+1's accumulation/stats don't serialize behind
            # rep r's finalization (cross-rep overlap; no-op for n_reps=1)
            opool=ctx.enter_context(tc.tile_pool(name="opool", bufs=2)),
            stats=ctx.enter_context(tc.tile_pool(name="stats", bufs=2)),
            ps_kb=ctx.enter_context(tc.tile_pool(name="ps_kb", bufs=1, space="PSUM")),
            ps_bb1=ctx.enter_context(tc.tile_pool(name="ps_bb1", bufs=1, space="PSUM")),
            ps_bb2=ctx.enter_context(tc.tile_pool(name="ps_bb2", bufs=1, space="PSUM")),
            ps_q=ctx.enter_context(tc.tile_pool(name="ps_q", bufs=1, space="PSUM")),
            ps_v=ctx.enter_context(tc.tile_pool(name="ps_v", bufs=1, space="PSUM")),
            ps_s=ctx.enter_context(tc.tile_pool(name="ps_s", bufs=1, space="PSUM")),
            ps_o=ctx.enter_context(tc.tile_pool(name="ps_o", bufs=1, space="PSUM")),
        )

        def emit_rep(rep, deferred):
            """Emit one rep; returns this rep's deferred b1 finalization.

            deferred: previous rep's b1 finalization, emitted into THIS
            rep's stream after tile 3 so its AllGather wait is already
            satisfied when each engine SEQ reaches it.  A nested function
            so each rep's closures bind their own tiles (the rep loop
            would otherwise rebind shared locals under the deferred call).
            """
            p = per_rep_pools
            out_acc = p["opool"].tile([C, B * NVOX], f32, tag="out_acc")
            # per-tile stats, one tile: col b*NTILES+t = sums, col
            # B*NTILES + b*NTILES + t = sums-of-squares (so each batch's
            # sums and ssqs reduce in ONE strided TensorReduce)
            BN = B * NTILES
            st2 = p["stats"].tile([C, 2 * BN], f32, tag="st2")
            dump = p["stats"].tile([C, NT], f32, tag="dump")

            tiles = [(b, t) for b in range(B) for t in range(NTILES)]
            xres_b = {}
            ychunk_state = {}
            fstate = {}
            sstate = {}

            def load_x(b):
                if b in xres_b:
                    return
                xr = p["xres"].tile([C, NVOX], bf16, tag="xres")
                nc.sync.dma_start(xr[:], x_io[b])
                xres_b[b] = xr

            def load_ychunk(b, ci):
                if (b, ci) in ychunk_state:
                    return
                yc = p["ypool"].tile([C, COND * 4 * NT], bf16, tag="ychunk")
                ysrc = y_io[b].rearrange("j c v -> c j v")
                # per-cond DMAs: K-proj for cond j only waits on its slice
                for j in range(COND):
                    nc.sync.dma_start(
                        yc[:, j * 4 * NT: (j + 1) * 4 * NT],
                        ysrc[:, bass.ts(j, 1), bass.ts(ci, 4 * NT)]
                        .rearrange("p j v -> p (j v)"),
                    )
                ychunk_state[(b, ci)] = yc

            def front1(k):
                """DMAs, Q proj, K h0 proj, V proj+copies."""
                b, t = tiles[k]
                load_x(b)
                load_ychunk(b, t // 4)
                ychunk = ychunk_state[(b, t // 4)]
                yj_of = lambda j: ychunk[:, j * 4 * NT + (t % 4) * NT:
                                         j * 4 * NT + (t % 4 + 1) * NT]
                xt = xres_b[b][:, ts(t, NT)]
                psQ = p["ps_q"].tile([C, NT], f32, tag="psq")
                mm(psQ[:], wT["wq"], xt)
                qsb = p["sb"].tile([C, NT], bf16, tag="qsb")
                nc.scalar.activation(qsb[:], psQ[:], Act.Identity,
                                     bias=vecs["bq"])
                psKB0 = p["ps_kb"].tile([C, 2 * NT], f32, tag="kb")
                for j in (0, 1):
                    mm(psKB0[:, ts(j, NT)], wT["wk"], yj_of(j))
                vbig = p["big"].tile([C, COND * NT], f32, tag="vbig")
                for j in range(COND):
                    psV = p["ps_v"].tile([C, NT], f32, tag="psv")
                    mm(psV[:], wT["wv"], yj_of(j))
                    nc.scalar.activation(vbig[:, ts(j, NT)], psV[:],
                                         Act.Identity, bias=vecs["bv"])
                qkbig = p["big"].tile([C, COND * NT], bf16, tag="qkbig")
                psS = p["ps_s"].tile([32, NT], f32, tag="pss")
                fstate[k] = (psS, vbig, xt, qsb, qkbig, yj_of, psKB0)
                # prefetch the next y chunk / next batch's inputs during the
                # current chunk's idle DMA time (avoids a burst at t%4==0 and
                # at the batch boundary, where stores also compete)
                if t % 4 == 2:
                    if t // 4 + 1 < NTILES // 4:
                        load_ychunk(b, t // 4 + 1)
                    elif b + 1 < B:
                        load_x(b + 1)
                        load_ychunk(b + 1, 0)

            def front_qk(k, h):
                """QK mul half h + its score matmuls (+ K h1 projections).

                qk = k * q via a stride-0 broadcast of qsb over the 2-cond
                pair (one PSUM operand only: NCC_IBVF027).  bk is dropped
                entirely: within a head it shifts all 4 logits by the same
                q.bk, which softmax cancels exactly.
                """
                psS, vbig, xt, qsb, qkbig, yj_of, psKB = fstate[k]
                qrep = qsb[:].unsqueeze(1).broadcast_to([C, 2, NT])
                nc.vector.tensor_tensor(
                    qkbig[:, ts(h, 2 * NT)].rearrange("p (j v) -> p j v", j=2),
                    psKB[:].rearrange("p (j v) -> p j v", j=2),
                    qrep, Alu.mult)
                for j in (2 * h, 2 * h + 1):
                    mm(psS[:], mask32[:, ts(j, 32)], qkbig[:, ts(j, NT)],
                       start=(j == 0), stop=(j == COND - 1))
                if h == 0:
                    psKB1 = p["ps_kb"].tile([C, 2 * NT], f32, tag="kb")
                    for j in (2, 3):
                        mm(psKB1[:, ts(j - 2, NT)], wT["wk"], yj_of(j))
                    fstate[k] = (psS, vbig, xt, qsb, qkbig, yj_of, psKB1)

            def soft(k):
                """exp, Z-matmul, reciprocal, E~ = E * (1/Z)."""
                b, t = tiles[k]
                psS, vbig, xt, qsb, qkbig, yj_of, _ = fstate.pop(k)
                esb = p["sb"].tile([32, NT], f32r, tag="esb")
                nc.scalar.activation(esb[:], psS[:], Act.Exp, scale=0.25)
                # psZ lives in the ps_s pool: its WAR (next tile's scores)
                # sits later in the chain than ps_q's (next tile's Q-proj).
                psZ = p["ps_s"].tile([32, NT], f32, tag="pss")
                mm(psZ[:], lhsT32[:], esb[:])
                rsb = p["sb"].tile([32, NT], f32, tag="rsb")
                nc.vector.reciprocal(rsb[:], psZ[:])
                etsb = p["sb"].tile([32, NT], bf16, tag="etsb")
                nc.gpsimd.tensor_tensor(etsb[:], esb[:].bitcast(f32),
                                        rsb[:], Alu.mult)
                sstate[k] = (etsb, vbig, xt)

            def back_cond(k, j, wbig):
                """Broadcast matmul + attn*V multiply for one cond.

                Per-cond with two ping-ponged 1-bank PSUM pools: bcast j+2
                only waits on avm j, so the PE bcast hides under the DVE
                multiply of the previous cond."""
                etsb, vbig, xt = sstate[k]
                pool = p["ps_bb1"] if j % 2 == 0 else p["ps_bb2"]
                psB = pool.tile([C, NT], f32, tag="bb")
                mm(psB[:], maskb[:, ts(j, C)], etsb[:])
                nc.vector.tensor_tensor(
                    wbig[:, ts(j, NT)], psB[:],
                    vbig[:, ts(j, NT)], Alu.mult)

            def back_out(k, wbig):
                b, t = tiles[k]
                col = b * NTILES + t
                etsb, vbig, xt = sstate.pop(k)
                psO = p["ps_o"].tile([C, NT], f32, tag="pso")
                for j in range(COND):
                    mm(psO[:], wT["wo"], wbig[:, ts(j, NT)],
                       start=(j == 0), stop=False)
                # residual folded into the accumulation group: psO += I @ x
                mm(psO[:], ident[:], xt, start=False, stop=True)
                outt = out_acc[:, col * NT: (col + 1) * NT]
                nc.scalar.activation(
                    outt, psO[:], Act.Identity, bias=vecs["bo"],
                    accum_out=st2[:, col: col + 1])
                nc.scalar.activation(
                    dump[:], outt, Act.Square,
                    accum_out=st2[:, BN + col: BN + col + 1])

            cc_state = {}
            cc_sb = {}

            def gn_pre(b):
                """Reduce per-channel stats and launch the AllGather."""
                ccsb = p["stats"].tile([C, 2], f32, tag=f"ccsb{b}")
                # ONE strided reduce over [C, (2 stats, NTILES)] -> [C, 2]
                nc.vector.reduce_sum(
                    ccsb[:],
                    st2[:].rearrange("p (s x) -> p s x", s=2)
                    [:, :, b * NTILES:(b + 1) * NTILES],
                    axis=mybir.AxisListType.X)
                cc_in = dram.tile([C, 2], f32, tag=f"cc_in{b}")
                cc_out = dram.tile([NCORES, C, 2], f32, tag=f"cc_out{b}")
                # Act HWDGE queue: must not sit behind bulk stores on SP
                nc.scalar.dma_start(cc_in[:], ccsb[:])
                nc.gpsimd.collective_compute(
                    "AllGather", Alu.bypass,
                    replica_groups=[list(range(NCORES))],
                    ins=[cc_in.opt()], outs=[cc_out.opt()])
                cc_state[b] = cc_out
                cc_sb[b] = ccsb

            def gn_post(b):
                """Stats -> per-channel affine -> rescale out_acc -> store."""
                cc_out = cc_state.pop(b)
                # gather the 8 cores' [C,2] partials: gs16[:, 0:8]=sums,
                # gs16[:, 8:16]=ssqs (s-major so the reduces are contiguous)
                gs16 = p["stats"].tile([C, 16], f32, tag=f"gs16_{b}")
                nc.sync.dma_start(
                    gs16[:].rearrange("p (s n) -> p s n", n=NCORES),
                    cc_out[:].rearrange("n p s -> p s n"))
                # one reduce over cores -> per-channel sums [C, 2]
                gsb = p["stats"].tile([C, 2], f32, tag=f"gsb{b}")
                nc.vector.reduce_sum(
                    gsb[:], gs16[:].rearrange("p (s n) -> p s n", n=NCORES),
                    axis=mybir.AxisListType.X)
                # fused group reduce+broadcast+mean: psGB = [mean, E[x^2]].
                # The ps_q WAR this creates is harmless: for b=1 this op is
                # emitted after the NEXT rep's tile 3 (deferred), so the
                # next rep's early Q-projections precede it in the stream.
                psGB = p["ps_q"].tile([C, 2], f32, tag="psq")
                nc.tensor.matmul(psGB[:], lhsT=ggmask[:], rhs=gsb[:],
                                 start=True, stop=True)
                msb = p["stats"].tile([C, 2], f32, tag=f"msb{b}")
                nc.vector.tensor_copy(msb[:], psGB[:])
                # negvar = mean^2 - E[x^2]; rstd = 1/sqrt(eps - negvar)
                nvar = p["stats"].tile([C, 1], f32, tag=f"nvar{b}")
                nc.vector.scalar_tensor_tensor(
                    nvar[:], msb[:, 0:1], msb[:, 0:1], msb[:, 1:2],
                    Alu.mult, Alu.subtract)
                sstd = p["stats"].tile([C, 1], f32, tag=f"sstd{b}")
                nc.scalar.activation(sstd[:], nvar[:], Act.Sqrt,
                                     bias=eps_t[:], scale=-1.0)
                rstd = p["stats"].tile([C, 1], f32, tag=f"rstd{b}")
                nc.vector.reciprocal(rstd[:], sstd[:])
                scale_b = p["stats"].tile([C, 1], f32, tag=f"scale{b}")
                nc.vector.tensor_tensor(scale_b[:], rstd[:],
                                        vecs["gamma"], Alu.mult)
                gate = cc_sb.get(1 - b) if b == 0 else None
                if gate is not None:
                    # 0*ccsb(b1) + scale: a zero-contribution data dependency
                    # that stops the scheduler hoisting b0's rescale+stores
                    # out of the b1 AllGather window they are meant to fill
                    gated = p["stats"].tile([C, 1], f32, tag=f"gsc{b}")
                    nc.vector.scalar_tensor_tensor(
                        gated[:], gate[:, 0:1], 0.0, scale_b[:],
                        Alu.mult, Alu.add)
                    scale_b = gated
                negb_b = p["stats"].tile([C, 2], f32, tag=f"negb{b}")
                nc.vector.scalar_tensor_tensor(
                    negb_b[:, 0:1], msb[:, 0:1], scale_b[:],
                    vecs["beta"], Alu.mult, Alu.subtract)
                nc.vector.tensor_scalar(negb_b[:, 1:2], negb_b[:, 0:1],
                                        -1.0, None, Alu.mult)
                # rescale tile-PAIRS split over three engines (DVE 2x all-
                # SBUF mode / Act / Pool), storing each pair once rescaled
                for tp in range(NTILES // 2):
                    src = out_acc[:, (b * NTILES + 2 * tp) * NT:
                                  (b * NTILES + 2 * tp + 2) * NT]
                    if tp >= 2:
                        nc.gpsimd.tensor_scalar(
                            src, src,
                            scale_b[:], negb_b[:, 0:1], Alu.mult, Alu.subtract)
                    elif tp == 1:
                        nc.scalar.activation(src, src, Act.Identity,
                                             scale=scale_b[:],
                                             bias=negb_b[:, 1:2])
                    else:
                        nc.vector.tensor_scalar(
                            src, src,
                            scale_b[:], negb_b[:, 0:1], Alu.mult, Alu.subtract)
                    nc.sync.dma_start(out_io[b][:, ts(tp, 2 * NT)], src)

            NK = len(tiles)
            for k in range(NK + 2):
                if k == 6 and deferred is not None:
                    deferred()
                    deferred = None
                if 1 <= k <= NK:
                    soft(k - 1)
                if k >= 2:
                    wbig = p["big"].tile([C, COND * NT], f32r, tag="qkbig")
                    back_cond(k - 2, 0, wbig)
                    back_cond(k - 2, 1, wbig)
                if k < NK:
                    front1(k)
                    front_qk(k, 0)
                if k >= 2:
                    back_cond(k - 2, 2, wbig)
                    back_cond(k - 2, 3, wbig)
                if k < NK:
                    front_qk(k, 1)
                if k >= 2:
                    back_out(k - 2, wbig)
                    bdone, tdone = tiles[k - 2]
                    if tdone == NTILES - 1:
                        gn_pre(bdone)
            # gn_post(0) emits now: it depends only on the (long-finished)
            # b0 AllGather and its gated rescale+stores fill the b1
            # AllGather latency.  gn_post(1) is deferred into the NEXT
            # rep's stream (emitted after its tile 3) so the b1 AllGather
            # wait never blocks an engine SEQ ahead of next-rep work.
            gn_post(0)
            return lambda: gn_post(1)

        pending = None
        for rep in range(n_reps):
            pending = emit_rep(rep, pending)
        pending()


    _split_waits(nc)
    return nc


def _shard_inputs(inputs):
    import ml_dtypes
    bf16 = ml_dtypes.bfloat16
    x = np.asarray(inputs["decoder_features"], np.float32).astype(bf16)
    y = np.asarray(inputs["skip_connection_features"], np.float32).astype(bf16)

    def wT(name, dtype):
        w = np.asarray(inputs[name], np.float32)
        return np.ascontiguousarray(w.T).astype(dtype)

    wqkv = np.concatenate([wT("w_q", bf16), wT("w_k", bf16),
                           wT("w_v", bf16)], axis=1)
    vec6 = np.stack([np.asarray(inputs[n], np.float32) for n in
                     ("b_q", "b_k", "b_v", "b_o", "gn_gamma", "gn_beta")],
                    axis=1)
    base = {
        "wqkv": np.ascontiguousarray(wqkv),
        "wo": wT("w_o", np.float32),
        "vec6": np.ascontiguousarray(vec6),
    }
    in_maps = []
    for ci in range(NCORES):
        sl = slice(HS * ci, HS * (ci + 1))
        im = dict(base)
        im["x"] = np.ascontiguousarray(x[:, :, sl]).reshape(B, C, NVOX)
        im["y"] = np.ascontiguousarray(y[:, :, :, sl]).reshape(B, COND, C, NVOX)
        in_maps.append(im)
    return in_maps


class _Runner:
    """Persistent PJRT runner: trace/compile once, execute many times.

    Mirrors concourse.bass2jax.run_bass_via_pjrt's multi-core branch but
    keeps the jitted shard_map callable alive so repeat calls skip
    re-tracing and NEFF recompilation.
    """

    def __init__(self, nc, donate=True):
        import jax
        from jax.sharding import Mesh, PartitionSpec
        from jax.experimental.shard_map import shard_map
        from concourse import bass2jax, mybir

        bass2jax.install_neuronx_cc_hook()
        assert nc.dbg_addr is None
        partition_name = (nc.partition_id_tensor.name
                          if nc.partition_id_tensor else None)
        in_names, out_names, out_avals, zero_outs = [], [], [], []
        for alloc in nc.m.functions[0].allocations:
            if not isinstance(alloc, mybir.MemoryLocationSet):
                continue
            name = alloc.memorylocations[0].name
            if alloc.kind == "ExternalInput":
                if name != partition_name:
                    in_names.append(name)
            elif alloc.kind == "ExternalOutput":
                out_names.append(name)
                shape = tuple(alloc.tensor_shape)
                dtype = mybir.dt.np(alloc.dtype)
                out_avals.append(jax.core.ShapedArray(shape, dtype))
                zero_outs.append(np.zeros(shape, dtype))
        n_params = len(in_names)
        n_outs = len(out_avals)
        in_names.extend(out_names)
        if partition_name is not None:
            in_names.append(partition_name)
        donate_idx = tuple(range(n_params, n_params + n_outs)) if donate else ()

        def _body(*args):
            operands = list(args)
            if partition_name is not None:
                operands.append(bass2jax.partition_id_tensor())
            outs = bass2jax._bass_exec_p.bind(
                *operands,
                out_avals=tuple(out_avals),
                in_names=tuple(in_names),
                out_names=tuple(out_names),
                lowering_input_output_aliases=(),
                sim_require_finite=True,
                sim_require_nnan=True,
                nc=nc,
            )
            return tuple(outs)

        devices = jax.devices()[:NCORES]
        mesh = Mesh(np.asarray(devices), ("core",))
        in_specs = (PartitionSpec("core"),) * (n_params + n_outs)
        out_specs = (PartitionSpec("core"),) * n_outs
        self._fn = jax.jit(
            shard_map(_body, mesh=mesh, in_specs=in_specs,
                      out_specs=out_specs, check_rep=False),
            donate_argnums=donate_idx, keep_unused=True)
        self._in_names = in_names[:n_params]
        self._out_names = out_names
        self._out_avals = out_avals
        self._zero_outs = zero_outs
        self._jax = jax

    def __call__(self, in_maps):
        concat_in = [
            np.concatenate([np.asarray(m[name]) for m in in_maps], axis=0)
            for name in self._in_names
        ]
        concat_zeros = [
            np.zeros((NCORES * z.shape[0], *z.shape[1:]), z.dtype)
            for z in self._zero_outs
        ]
        out_arrs = self._fn(*concat_in, *concat_zeros)
        out_arrs = self._jax.block_until_ready(out_arrs)
        return [
            {
                name: np.asarray(out_arrs[i]).reshape(
                    NCORES, *self._out_avals[i].shape)[c]
                for i, name in enumerate(self._out_names)
            }
            for c in range(NCORES)
        ]


class _Results:
    def __init__(self, results):
        self.results = results


def _get_runner(n_reps=1, donate=True):
    key = (n_reps, donate)
    if key not in _CACHE:
        _CACHE[key] = _Runner(_build(n_reps), donate=donate)
    return _CACHE[key]


def _run(in_maps, n_reps=1):
    return _Results(_get_runner(n_reps)(in_maps))


def kernel(**inputs) -> np.ndarray:
    res = _run(_shard_inputs(inputs))
    out = np.empty((B, C, H, W, D), np.float32)
    for ci in range(NCORES):
        sl = slice(HS * ci, HS * (ci + 1))
        out[:, :, sl] = res.results[ci]["out"].reshape(B, C, HS, W, D)
    return out



# revision 65
# speedup vs baseline: 32.4591x; 1.0104x over previous
"""Trainium2 Bass kernel for nn_DecoderCrossAttention.

Reference computation (per voxel v, batch b):
    q = Wq x_v + bq                        (x = decoder_features, [C])
    k_j = Wk y_jv + bk, v_j = Wv y_jv + bv (y = skip features, COND=4 frames)
    s_j[h] = <q_h, k_jh> / sqrt(DH)        (NH=8 heads of DH=16)
    attn = softmax_j(s)                    (over the 4 conditioning frames)
    o = Wo (sum_j attn_j * v_j) + bo + x_v
    out = GroupNorm8(o) * gamma + beta     (stats over (C/G, H, W, D) per batch)

Strategy (8 NeuronCores, data-parallel over H):
  * Each core gets H-slice of 4 planes: 2*4*32*32 = 8192 voxels.
  * Feature-major layout [C=128 partitions, voxels in free dim], 512-voxel tiles.
  * x/y/Wq/Wk/Wv in bf16 (host-cast, host-transposed weights): halves input
    DMA; rel-err gate is 2e-2, measured ~2.4e-3.
  * Per-head score reduction (sum over the 16 channels of a head) and the
    softmax broadcast (8 head rows -> 128 channels) are PE matmuls against
    0/1 masks built in-kernel with iota+compare.
  * Softmax over only 4 logits, inputs are bounded => no max subtraction.
  * E~ = exp(s)*recip(Z): exp on Act, recip on DVE, the product on Pool.
  * qk = k*q via a stride-0 broadcast AP of q over the 2-cond pair (one
    PSUM operand only - NCC_IBVF027).  bk is dropped entirely: within a
    head it shifts all 4 logits by the same q.bk, which softmax cancels
    exactly.
  * attn*V products (per-cond, through two ping-ponged 1-bank PSUM pools
    so the PE broadcast hides under the previous cond's DVE multiply) feed
    4 accumulating output-projection matmuls, the residual rides in as a
    5th identity matmul, and the Act engine evacuates psO with
    bias+per-channel sums (accum_out); ssq via Act Square.  (GPSIMD
    cannot access PSUM on HW, so every PSUM evacuation stays on Act/DVE.)
  * GroupNorm is global: per-channel sum/sumsq AllGather (15us fixed vs
    28us for AllReduce in the cost model) + local reduce; a
    zero-contribution stt gates the batch-0 finalization into the batch-1
    AllGather window.  Group stats are reduced+broadcast+meaned in ONE
    pre-scaled [C,C] group-mask matmul, variance via sqrt(scale=-1,
    bias=eps), and the rescale runs on tile-PAIRS, one on DVE and three
    on Pool (Act serializes the rep boundary; Pool idles there), with
    paired [C, 2*NT] stores.
  * Both per-tile stats live in ONE [C, 2*B*NTILES] tile (sums | ssqs)
    so each batch's AllGather input is a single strided TensorReduce.
  * THE key multi-rep optimization: each rep's batch-1 finalization
    (gather-DMA + stats chain + rescale + stores, all gated on that
    rep's 15us AllGather) is EMITTED INTO THE NEXT REP'S instruction
    stream, after its tile 5.  The engine SEQs are in-order, so a wait
    placed at the end of rep r would block every engine's rep r+1 work
    behind the collective; deferred emission lets rep r+1's front run
    during rep r's AllGather window, cutting the marginal per-rep time
    from 134.3us to 109.9us in the TimelineSim cost model.
  * Constants are packed into 3 DMAs (wqkv, wo, vec6) and y is fetched
    per-cond with a 2-tile prefetch: first matmul fires at ~5us.

The walrus build here accepts only ONE sync wait per instruction; Tile
attaches many.  split_waits() hoists extras onto standalone EventSemaphore
instructions post-scheduling.
"""

import sys

if "/opt/trn_rl_repo" not in sys.path:
    sys.path.insert(0, "/opt/trn_rl_repo")

import numpy as np

B, COND, C, H, W, D = 2, 4, 128, 32, 32, 32
NH, DH, G = 8, 16, 8
EPS = 1e-5
NCORES = 8
HS = H // NCORES          # 4 H-planes per core
NVOX = HS * W * D         # 4096 voxels per batch per core
NT = 512                  # voxels per tile
NTILES = NVOX // NT       # 8 tiles per batch
N_GROUP = (C // G) * H * W * D   # elements per (batch, group) for GN stats

_CACHE = {}


def _split_waits(nc):
    """Hoist extra sync waits onto standalone EventSemaphore instructions."""
    from concourse import mybir
    import bass_rust

    n_split = 0
    for func in nc.m.functions:
        for blk in func.blocks:
            new_list = []
            changed = False
            for inst in blk.instructions:
                si = inst.sync_info
                waits = list(si.on_wait) if si is not None else []
                if len(waits) > 1:
                    changed = True
                    for w in waits[:-1]:
                        ev = mybir.InstEventSemaphore(
                            name=f"wsplit-{nc.next_id()}", ins=[], outs=[]
                        )
                        ev.engine = inst.engine
                        ev.sync_info = bass_rust.SyncInfo(on_wait=[w], on_update=[])
                        new_list.append(ev)
                        n_split += 1
                    inst.sync_info = bass_rust.SyncInfo(
                        on_wait=[waits[-1]], on_update=list(si.on_update)
                    )
                new_list.append(inst)
            if changed:
                blk.instructions = new_list
    return n_split


def _build(n_reps=1):
    import concourse.bass as bass
    import concourse.tile as tile
    from concourse import mybir
    from concourse.bass_isa import ReduceOp
    from contextlib import ExitStack

    dt = mybir.dt
    f32 = dt.float32
    f32r = dt.float32r
    i32 = dt.int32
    Alu = mybir.AluOpType
    Act = mybir.ActivationFunctionType
    ts = bass.ts

    bf16 = dt.bfloat16

    nc = bass.Bass("TRN2", target_bir_lowering=False, debug=False,
                   num_devices=NCORES)
    x_io = nc.dram_tensor("x", [B, C, NVOX], bf16, kind="ExternalInput").ap()
    y_io = nc.dram_tensor("y", [B, COND, C, NVOX], bf16, kind="ExternalInput").ap()
    # constants packed into 3 tensors: each dma_start costs ~0.6us of HWDGE
    # issue time, so 10 separate loads would delay the first x/y input DMAs
    wqkv_io = nc.dram_tensor("wqkv", [C, 3 * C], bf16, kind="ExternalInput").ap()
    wo_io = nc.dram_tensor("wo", [C, C], f32r, kind="ExternalInput").ap()
    vec6_io = nc.dram_tensor("vec6", [C, 6], f32, kind="ExternalInput").ap()
    out_io = nc.dram_tensor("out", [B, C, NVOX], f32, kind="ExternalOutput").ap()

    def mm(out, lhsT, rhs, start=True, stop=True):
        nc.tensor.matmul(out, lhsT=lhsT, rhs=rhs, start=start, stop=stop)

    with tile.TileContext(nc) as tc, ExitStack() as ctx:
        # ---------------- constants / weights / masks -------------------
        const = ctx.enter_context(tc.tile_pool(name="const", bufs=1))
        dram = ctx.enter_context(tc.tile_pool(name="dram", bufs=1, space="DRAM"))

        # constants first, on the Activation HWDGE queue, packed: 3 issues
        wqkv = const.tile([C, 3 * C], bf16, tag="wqkv")
        nc.scalar.dma_start(wqkv[:], wqkv_io[:])
        wo_t = const.tile([C, C], f32r, tag="wT_wo")
        nc.scalar.dma_start(wo_t[:], wo_io[:])
        vec6 = const.tile([C, 6], f32, tag="vec6")
        nc.scalar.dma_start(vec6[:], vec6_io[:])
        wT = {"wq": wqkv[:, 0:C], "wk": wqkv[:, C:2 * C],
              "wv": wqkv[:, 2 * C:3 * C], "wo": wo_t[:]}
        vecs = {name: vec6[:, i:i + 1] for i, name in
                enumerate(("bq", "bk", "bv", "bo", "gamma", "beta"))}

        # --- masks via iota + compare (int32), cast to f32
        with tc.tile_pool(name="setup", bufs=1) as setup:
            def icast(dst_ap, src_ap):
                nc.vector.tensor_copy(dst_ap, src_ap)

            # partition-index and free-index helpers
            p128 = setup.tile([C, C], i32, tag="p128")
            nc.gpsimd.iota(p128[:], pattern=[[0, C]], base=0, channel_multiplier=1)
            f128 = setup.tile([C, C], i32, tag="f128")
            nc.gpsimd.iota(f128[:], pattern=[[1, C]], base=0, channel_multiplier=0)
            hc128 = setup.tile([C, C], i32, tag="hc128")
            nc.vector.tensor_scalar(hc128[:], p128[:], 4, None,
                                    Alu.arith_shift_right)
            tmpi = setup.tile([C, C], i32, tag="tmpi")

            # identity [128,128] (bf16, for the residual pass-through matmul)
            ident = const.tile([C, C], bf16, tag="ident")
            nc.vector.tensor_tensor(tmpi[:], f128[:], p128[:], Alu.is_equal)
            icast(ident[:], tmpi[:])

            # mask32 [128, 4*32]: col 32j+m ; 1 iff (m - 8j) == c//16
            jm = setup.tile([C, C], i32, tag="jm")
            nc.gpsimd.iota(jm[:].rearrange("p (j m) -> p j m", j=4),
                           pattern=[[-8, 4], [1, 32]], base=0,
                           channel_multiplier=0)
            mask32 = const.tile([C, C], bf16, tag="mask32")
            nc.vector.tensor_tensor(tmpi[:], jm[:], hc128[:], Alu.is_equal)
            icast(mask32[:], tmpi[:])

            # lhsT32 [32,32]: 1 iff p%8 == m%8  (Z replication matmul)
            p32 = setup.tile([32, 32], i32, tag="p32")
            nc.gpsimd.iota(p32[:], pattern=[[0, 32]], base=0, channel_multiplier=1)
            pm32 = setup.tile([32, 32], i32, tag="pm32")
            nc.vector.tensor_scalar(pm32[:], p32[:], 3, 3,
                                    Alu.arith_shift_right, Alu.arith_shift_left)
            t32 = setup.tile([32, 32], i32, tag="t32")
            nc.vector.tensor_tensor(t32[:], p32[:], pm32[:], Alu.subtract)
            fm32 = setup.tile([32, 32], i32, tag="fm32")
            nc.gpsimd.iota(fm32[:].rearrange("p (j m) -> p j m", j=4),
                           pattern=[[0, 4], [1, 8]], base=0, channel_multiplier=0)
            e32 = setup.tile([32, 32], i32, tag="e32")
            nc.vector.tensor_tensor(e32[:], fm32[:], t32[:], Alu.is_equal)
            lhsT32 = const.tile([32, 32], f32r, tag="lhsT32")
            icast(lhsT32[:], e32[:])

            # maskb [32, 4*128]: col 128j+c ; 1 iff (p - 8j) == c//16
            pj = setup.tile([32, 4 * C], i32, tag="pj")
            nc.gpsimd.iota(pj[:].rearrange("p (j c) -> p j c", j=4),
                           pattern=[[-8, 4], [0, C]], base=0,
                           channel_multiplier=1)
            fc = setup.tile([32, 4 * C], i32, tag="fc")
            nc.gpsimd.iota(fc[:].rearrange("p (j c) -> p j c", j=4),
                           pattern=[[0, 4], [1, C]], base=0, channel_multiplier=0)
            nc.vector.tensor_scalar(fc[:], fc[:], 4, None, Alu.arith_shift_right)
            eb = setup.tile([32, 4 * C], i32, tag="eb")
            nc.vector.tensor_tensor(eb[:], pj[:], fc[:], Alu.is_equal)
            maskb = const.tile([32, 4 * C], bf16, tag="maskb")
            icast(maskb[:], eb[:])

            # ggmask [128, 128]: 1/N_GROUP iff p//16 == c//16  (GN group sum,
            # fused reduce+broadcast+mean: psGB = ggmask.T @ stats gives the
            # group means [mean, E[x^2]] directly at channel layout)
            fg = setup.tile([C, C], i32, tag="fg")
            nc.vector.tensor_scalar(fg[:], f128[:], 4, None,
                                    Alu.arith_shift_right)
            egg = setup.tile([C, C], i32, tag="egg")
            nc.vector.tensor_tensor(egg[:], fg[:], hc128[:], Alu.is_equal)
            ggmask = const.tile([C, C], f32, tag="ggmask")
            icast(ggmask[:], egg[:])
            nc.vector.tensor_scalar(ggmask[:], ggmask[:], 1.0 / N_GROUP, None,
                                    Alu.mult)
            eps_t = const.tile([C, 1], f32, tag="eps_t")
            nc.vector.memset(eps_t[:], EPS)

        # ---------------- main pipeline ---------------------------------
        per_rep_pools = dict(
            xres=ctx.enter_context(tc.tile_pool(name="xres", bufs=2)),
            ypool=ctx.enter_context(tc.tile_pool(name="ypool", bufs=2)),
            sb=ctx.enter_context(tc.tile_pool(name="sb", bufs=2)),
            big=ctx.enter_context(tc.tile_pool(name="bigsb", bufs=2)),
            # bufs=2 so rep r+1's accumulation/stats don't serialize behind
            # rep r's finalization (cross-rep overlap; no-op for n_reps=1)
            opool=ctx.enter_context(tc.tile_pool(name="opool", bufs=2)),
            stats=ctx.enter_context(tc.tile_pool(name="stats", bufs=2)),
            ps_kb=ctx.enter_context(tc.tile_pool(name="ps_kb", bufs=1, space="PSUM")),
            ps_bb1=ctx.enter_context(tc.tile_pool(name="ps_bb1", bufs=1, space="PSUM")),
            ps_bb2=ctx.enter_context(tc.tile_pool(name="ps_bb2", bufs=1, space="PSUM")),
            ps_q=ctx.enter_context(tc.tile_pool(name="ps_q", bufs=1, space="PSUM")),
            ps_v=ctx.enter_context(tc.tile_pool(name="ps_v", bufs=1, space="PSUM")),
            ps_s=ctx.enter_context(tc.tile_pool(name="ps_s", bufs=1, space="PSUM")),
            ps_o=ctx.enter_context(tc.tile_pool(name="ps_o", bufs=1, space="PSUM")),
        )

        def emit_rep(rep, deferred):
            """Emit one rep; returns this rep's deferred b1 finalization.

            deferred: previous rep's b1 finalization, emitted into THIS
            rep's stream after tile 3 so its AllGather wait is already
            satisfied when each engine SEQ reaches it.  A nested function
            so each rep's closures bind their own tiles (the rep loop
            would otherwise rebind shared locals under the deferred call).
            """
            p = per_rep_pools
            out_acc = p["opool"].tile([C, B * NVOX], f32, tag="out_acc")
            # per-tile stats, one tile: col b*NTILES+t = sums, col
            # B*NTILES + b*NTILES + t = sums-of-squares (so each batch's
            # sums and ssqs reduce in ONE strided TensorReduce)
            BN = B * NTILES
            st2 = p["stats"].tile([C, 2 * BN], f32, tag="st2")
            dump = p["stats"].tile([C, NT], f32, tag="dump")

            tiles = [(b, t) for b in range(B) for t in range(NTILES)]
            xres_b = {}
            ychunk_state = {}
            fstate = {}
            sstate = {}

            def load_x(b):
                if b in xres_b:
                    return
                xr = p["xres"].tile([C, NVOX], bf16, tag="xres")
                nc.sync.dma_start(xr[:], x_io[b])
                xres_b[b] = xr

            def load_ychunk(b, ci):
                if (b, ci) in ychunk_state:
                    return
                yc = p["ypool"].tile([C, COND * 4 * NT], bf16, tag="ychunk")
                ysrc = y_io[b].rearrange("j c v -> c j v")
                # per-cond DMAs: K-proj for cond j only waits on its slice
                for j in range(COND):
                    nc.sync.dma_start(
                        yc[:, j * 4 * NT: (j + 1) * 4 * NT],
                        ysrc[:, bass.ts(j, 1), bass.ts(ci, 4 * NT)]
                        .rearrange("p j v -> p (j v)"),
                    )
                ychunk_state[(b, ci)] = yc

            def front1(k):
                """DMAs, Q proj, K h0 proj, V proj+copies."""
                b, t = tiles[k]
                load_x(b)
                load_ychunk(b, t // 4)
                ychunk = ychunk_state[(b, t // 4)]
                yj_of = lambda j: ychunk[:, j * 4 * NT + (t % 4) * NT:
                                         j * 4 * NT + (t % 4 + 1) * NT]
                xt = xres_b[b][:, ts(t, NT)]
                psQ = p["ps_q"].tile([C, NT], f32, tag="psq")
                mm(psQ[:], wT["wq"], xt)
                qsb = p["sb"].tile([C, NT], bf16, tag="qsb")
                nc.scalar.activation(qsb[:], psQ[:], Act.Identity,
                                     bias=vecs["bq"])
                psKB0 = p["ps_kb"].tile([C, 2 * NT], f32, tag="kb")
                for j in (0, 1):
                    mm(psKB0[:, ts(j, NT)], wT["wk"], yj_of(j))
                vbig = p["big"].tile([C, COND * NT], f32, tag="vbig")
                for j in range(COND):
                    psV = p["ps_v"].tile([C, NT], f32, tag="psv")
                    mm(psV[:], wT["wv"], yj_of(j))
                    nc.scalar.activation(vbig[:, ts(j, NT)], psV[:],
                                         Act.Identity, bias=vecs["bv"])
                qkbig = p["big"].tile([C, COND * NT], bf16, tag="qkbig")
                psS = p["ps_s"].tile([32, NT], f32, tag="pss")
                fstate[k] = (psS, vbig, xt, qsb, qkbig, yj_of, psKB0)
                # prefetch the next y chunk / next batch's inputs during the
                # current chunk's idle DMA time (avoids a burst at t%4==0 and
                # at the batch boundary, where stores also compete)
                if t % 4 == 2:
                    if t // 4 + 1 < NTILES // 4:
                        load_ychunk(b, t // 4 + 1)
                    elif b + 1 < B:
                        load_x(b + 1)
                        load_ychunk(b + 1, 0)

            def front_qk(k, h):
                """QK mul half h + its score matmuls (+ K h1 projections).

                qk = k * q via a stride-0 broadcast of qsb over the 2-cond
                pair (one PSUM operand only: NCC_IBVF027).  bk is dropped
                entirely: within a head it shifts all 4 logits by the same
                q.bk, which softmax cancels exactly.
                """
                psS, vbig, xt, qsb, qkbig, yj_of, psKB = fstate[k]
                qrep = qsb[:].unsqueeze(1).broadcast_to([C, 2, NT])
                nc.vector.tensor_tensor(
                    qkbig[:, ts(h, 2 * NT)].rearrange("p (j v) -> p j v", j=2),
                    psKB[:].rearrange("p (j v) -> p j v", j=2),
                    qrep, Alu.mult)
                for j in (2 * h, 2 * h + 1):
                    mm(psS[:], mask32[:, ts(j, 32)], qkbig[:, ts(j, NT)],
                       start=(j == 0), stop=(j == COND - 1))
                if h == 0:
                    psKB1 = p["ps_kb"].tile([C, 2 * NT], f32, tag="kb")
                    for j in (2, 3):
                        mm(psKB1[:, ts(j - 2, NT)], wT["wk"], yj_of(j))
                    fstate[k] = (psS, vbig, xt, qsb, qkbig, yj_of, psKB1)

            def soft(k):
                """exp, Z-matmul, reciprocal, E~ = E * (1/Z)."""
                b, t = tiles[k]
                psS, vbig, xt, qsb, qkbig, yj_of, _ = fstate.pop(k)
                esb = p["sb"].tile([32, NT], f32r, tag="esb")
                nc.scalar.activation(esb[:], psS[:], Act.Exp, scale=0.25)
                # psZ lives in the ps_s pool: its WAR (next tile's scores)
                # sits later in the chain than ps_q's (next tile's Q-proj).
                psZ = p["ps_s"].tile([32, NT], f32, tag="pss")
                mm(psZ[:], lhsT32[:], esb[:])
                rsb = p["sb"].tile([32, NT], f32, tag="rsb")
                nc.vector.reciprocal(rsb[:], psZ[:])
                etsb = p["sb"].tile([32, NT], bf16, tag="etsb")
                nc.gpsimd.tensor_tensor(etsb[:], esb[:].bitcast(f32),
                                        rsb[:], Alu.mult)
                sstate[k] = (etsb, vbig, xt)

            def back_cond(k, j, wbig):
                """Broadcast matmul + attn*V multiply for one cond.

                Per-cond with two ping-ponged 1-bank PSUM pools: bcast j+2
                only waits on avm j, so the PE bcast hides under the DVE
                multiply of the previous cond."""
                etsb, vbig, xt = sstate[k]
                pool = p["ps_bb1"] if j % 2 == 0 else p["ps_bb2"]
                psB = pool.tile([C, NT], f32, tag="bb")
                mm(psB[:], maskb[:, ts(j, C)], etsb[:])
                nc.vector.tensor_tensor(
                    wbig[:, ts(j, NT)], psB[:],
                    vbig[:, ts(j, NT)], Alu.mult)

            def back_out(k, wbig):
                b, t = tiles[k]
                col = b * NTILES + t
                etsb, vbig, xt = sstate.pop(k)
                psO = p["ps_o"].tile([C, NT], f32, tag="pso")
                for j in range(COND):
                    mm(psO[:], wT["wo"], wbig[:, ts(j, NT)],
                       start=(j == 0), stop=False)
                # residual folded into the accumulation group: psO += I @ x
                mm(psO[:], ident[:], xt, start=False, stop=True)
                outt = out_acc[:, col * NT: (col + 1) * NT]
                nc.scalar.activation(
                    outt, psO[:], Act.Identity, bias=vecs["bo"],
                    accum_out=st2[:, col: col + 1])
                nc.scalar.activation(
                    dump[:], outt, Act.Square,
                    accum_out=st2[:, BN + col: BN + col + 1])

            cc_state = {}
            cc_sb = {}

            def gn_pre(b):
                """Reduce per-channel stats and launch the AllGather."""
                ccsb = p["stats"].tile([C, 2], f32, tag=f"ccsb{b}")
                # ONE strided reduce over [C, (2 stats, NTILES)] -> [C, 2]
                nc.vector.reduce_sum(
                    ccsb[:],
                    st2[:].rearrange("p (s x) -> p s x", s=2)
                    [:, :, b * NTILES:(b + 1) * NTILES],
                    axis=mybir.AxisListType.X)
                cc_in = dram.tile([C, 2], f32, tag=f"cc_in{b}")
                cc_out = dram.tile([NCORES, C, 2], f32, tag=f"cc_out{b}")
                # Act HWDGE queue: must not sit behind bulk stores on SP
                nc.scalar.dma_start(cc_in[:], ccsb[:])
                nc.gpsimd.collective_compute(
                    "AllGather", Alu.bypass,
                    replica_groups=[list(range(NCORES))],
                    ins=[cc_in.opt()], outs=[cc_out.opt()])
                cc_state[b] = cc_out
                cc_sb[b] = ccsb

            def gn_post(b):
                """Stats -> per-channel affine -> rescale out_acc -> store."""
                cc_out = cc_state.pop(b)
                # gather the 8 cores' [C,2] partials: gs16[:, 0:8]=sums,
                # gs16[:, 8:16]=ssqs (s-major so the reduces are contiguous)
                gs16 = p["stats"].tile([C, 16], f32, tag=f"gs16_{b}")
                nc.sync.dma_start(
                    gs16[:].rearrange("p (s n) -> p s n", n=NCORES),
                    cc_out[:].rearrange("n p s -> p s n"))
                # one reduce over cores -> per-channel sums [C, 2]
                gsb = p["stats"].tile([C, 2], f32, tag=f"gsb{b}")
                nc.vector.reduce_sum(
                    gsb[:], gs16[:].rearrange("p (s n) -> p s n", n=NCORES),
                    axis=mybir.AxisListType.X)
                # fused group reduce+broadcast+mean: psGB = [mean, E[x^2]].
                # The ps_q WAR this creates is harmless: for b=1 this op is
                # emitted after the NEXT rep's tile 3 (deferred), so the
                # next rep's early Q-projections precede it in the stream.
                psGB = p["ps_q"].tile([C, 2], f32, tag="psq")
                nc.tensor.matmul(psGB[:], lhsT=ggmask[:], rhs=gsb[:],
                                 start=True, stop=True)
                msb = p["stats"].tile([C, 2], f32, tag=f"msb{b}")
                nc.vector.tensor_copy(msb[:], psGB[:])
                # negvar = mean^2 - E[x^2]; rstd = 1/sqrt(eps - negvar)
                nvar = p["stats"].tile([C, 1], f32, tag=f"nvar{b}")
                nc.vector.scalar_tensor_tensor(
                    nvar[:], msb[:, 0:1], msb[:, 0:1], msb[:, 1:2],
                    Alu.mult, Alu.subtract)
                sstd = p["stats"].tile([C, 1], f32, tag=f"sstd{b}")
                nc.scalar.activation(sstd[:], nvar[:], Act.Sqrt,
                                     bias=eps_t[:], scale=-1.0)
                rstd = p["stats"].tile([C, 1], f32, tag=f"rstd{b}")
                nc.vector.reciprocal(rstd[:], sstd[:])
                scale_b = p["stats"].tile([C, 1], f32, tag=f"scale{b}")
                nc.vector.tensor_tensor(scale_b[:], rstd[:],
                                        vecs["gamma"], Alu.mult)
                gate = cc_sb.get(1 - b) if b == 0 else None
                if gate is not None:
                    # 0*ccsb(b1) + scale: a zero-contribution data dependency
                    # that stops the scheduler hoisting b0's rescale+stores
                    # out of the b1 AllGather window they are meant to fill
                    gated = p["stats"].tile([C, 1], f32, tag=f"gsc{b}")
                    nc.vector.scalar_tensor_tensor(
                        gated[:], gate[:, 0:1], 0.0, scale_b[:],
                        Alu.mult, Alu.add)
                    scale_b = gated
                negb_b = p["stats"].tile([C, 2], f32, tag=f"negb{b}")
                nc.vector.scalar_tensor_tensor(
                    negb_b[:, 0:1], msb[:, 0:1], scale_b[:],
                    vecs["beta"], Alu.mult, Alu.subtract)
                nc.vector.tensor_scalar(negb_b[:, 1:2], negb_b[:, 0:1],
                                        -1.0, None, Alu.mult)
                # rescale tile-PAIRS split over three engines (DVE 2x all-
                # SBUF mode / Act / Pool), storing each pair once rescaled
                for tp in range(NTILES // 2):
                    src = out_acc[:, (b * NTILES + 2 * tp) * NT:
                                  (b * NTILES + 2 * tp + 2) * NT]
                    if tp >= 1:
                        # Act is the rep-boundary serializer (next rep's
                        # qsb/V evacuations queue ahead of its first exp);
                        # keep the gated rescale off it entirely
                        nc.gpsimd.tensor_scalar(
                            src, src,
                            scale_b[:], negb_b[:, 0:1], Alu.mult, Alu.subtract)
                    else:
                        nc.vector.tensor_scalar(
                            src, src,
                            scale_b[:], negb_b[:, 0:1], Alu.mult, Alu.subtract)
                    nc.sync.dma_start(out_io[b][:, ts(tp, 2 * NT)], src)

            NK = len(tiles)
            for k in range(NK + 2):
                if k == 6 and deferred is not None:
                    deferred()
                    deferred = None
                if 1 <= k <= NK:
                    soft(k - 1)
                if k >= 2:
                    wbig = p["big"].tile([C, COND * NT], f32r, tag="qkbig")
                    back_cond(k - 2, 0, wbig)
                    back_cond(k - 2, 1, wbig)
                if k < NK:
                    front1(k)
                    front_qk(k, 0)
                if k >= 2:
                    back_cond(k - 2, 2, wbig)
                    back_cond(k - 2, 3, wbig)
                if k < NK:
                    front_qk(k, 1)
                if k >= 2:
                    back_out(k - 2, wbig)
                    bdone, tdone = tiles[k - 2]
                    if tdone == NTILES - 1:
                        gn_pre(bdone)
            # gn_post(0) emits now: it depends only on the (long-finished)
            # b0 AllGather and its gated rescale+stores fill the b1
            # AllGather latency.  gn_post(1) is deferred into the NEXT
            # rep's stream (emitted after its tile 3) so the b1 AllGather
            # wait never blocks an engine SEQ ahead of next-rep work.
            gn_post(0)
            return lambda: gn_post(1)

        pending = None
        for rep in range(n_reps):
            pending = emit_rep(rep, pending)
        pending()


    _split_waits(nc)
    return nc


def _shard_inputs(inputs):
    import ml_dtypes
    bf16 = ml_dtypes.bfloat16
    x = np.asarray(inputs["decoder_features"], np.float32).astype(bf16)
    y = np.asarray(inputs["skip_connection_features"], np.float32).astype(bf16)

    def wT(name, dtype):
        w = np.asarray(inputs[name], np.float32)
        return np.ascontiguousarray(w.T).astype(dtype)

    wqkv = np.concatenate([wT("w_q", bf16), wT("w_k", bf16),
                           wT("w_v", bf16)], axis=1)
    vec6 = np.stack([np.asarray(inputs[n], np.float32) for n in
                     ("b_q", "b_k", "b_v", "b_o", "gn_gamma", "gn_beta")],
                    axis=1)
    base = {
        "wqkv": np.ascontiguousarray(wqkv),
        "wo": wT("w_o", np.float32),
        "vec6": np.ascontiguousarray(vec6),
    }
    in_maps = []
    for ci in range(NCORES):
        sl = slice(HS * ci, HS * (ci + 1))
        im = dict(base)
        im["x"] = np.ascontiguousarray(x[:, :, sl]).reshape(B, C, NVOX)
        im["y"] = np.ascontiguousarray(y[:, :, :, sl]).reshape(B, COND, C, NVOX)
        in_maps.append(im)
    return in_maps


class _Runner:
    """Persistent PJRT runner: trace/compile once, execute many times.

    Mirrors concourse.bass2jax.run_bass_via_pjrt's multi-core branch but
    keeps the jitted shard_map callable alive so repeat calls skip
    re-tracing and NEFF recompilation.
    """

    def __init__(self, nc, donate=True):
        import jax
        from jax.sharding import Mesh, PartitionSpec
        from jax.experimental.shard_map import shard_map
        from concourse import bass2jax, mybir

        bass2jax.install_neuronx_cc_hook()
        assert nc.dbg_addr is None
        partition_name = (nc.partition_id_tensor.name
                          if nc.partition_id_tensor else None)
        in_names, out_names, out_avals, zero_outs = [], [], [], []
        for alloc in nc.m.functions[0].allocations:
            if not isinstance(alloc, mybir.MemoryLocationSet):
                continue
            name = alloc.memorylocations[0].name
            if alloc.kind == "ExternalInput":
                if name != partition_name:
                    in_names.append(name)
            elif alloc.kind == "ExternalOutput":
                out_names.append(name)
                shape = tuple(alloc.tensor_shape)
                dtype = mybir.dt.np(alloc.dtype)
                out_avals.append(jax.core.ShapedArray(shape, dtype))
                zero_outs.append(np.zeros(shape, dtype))
        n_params = len(in_names)
        n_outs = len(out_avals)
        in_names.extend(out_names)
        if partition_name is not None:
            in_names.append(partition_name)
        donate_idx = tuple(range(n_params, n_params + n_outs)) if donate else ()

        def _body(*args):
            operands = list(args)
            if partition_name is not None:
                operands.append(bass2jax.partition_id_tensor())
            outs = bass2jax._bass_exec_p.bind(
                *operands,
                out_avals=tuple(out_avals),
                in_names=tuple(in_names),
                out_names=tuple(out_names),
                lowering_input_output_aliases=(),
                sim_require_finite=True,
                sim_require_nnan=True,
                nc=nc,
            )
            return tuple(outs)

        devices = jax.devices()[:NCORES]
        mesh = Mesh(np.asarray(devices), ("core",))
        in_specs = (PartitionSpec("core"),) * (n_params + n_outs)
        out_specs = (PartitionSpec("core"),) * n_outs
        self._fn = jax.jit(
            shard_map(_body, mesh=mesh, in_specs=in_specs,
                      out_specs=out_specs, check_rep=False),
            donate_argnums=donate_idx, keep_unused=True)
        self._in_names = in_names[:n_params]
        self._out_names = out_names
        self._out_avals = out_avals
        self._zero_outs = zero_outs
        self._jax = jax

    def __call__(self, in_maps):
        concat_in = [
            np.concatenate([np.asarray(m[name]) for m in in_maps], axis=0)
            for name in self._in_names
        ]
        concat_zeros = [
            np.zeros((NCORES * z.shape[0], *z.shape[1:]), z.dtype)
            for z in self._zero_outs
        ]
        out_arrs = self._fn(*concat_in, *concat_zeros)
        out_arrs = self._jax.block_until_ready(out_arrs)
        return [
            {
                name: np.asarray(out_arrs[i]).reshape(
                    NCORES, *self._out_avals[i].shape)[c]
                for i, name in enumerate(self._out_names)
            }
            for c in range(NCORES)
        ]


class _Results:
    def __init__(self, results):
        self.results = results


def _get_runner(n_reps=1, donate=True):
    key = (n_reps, donate)
    if key not in _CACHE:
        _CACHE[key] = _Runner(_build(n_reps), donate=donate)
    return _CACHE[key]


def _run(in_maps, n_reps=1):
    return _Results(_get_runner(n_reps)(in_maps))


def kernel(**inputs) -> np.ndarray:
    res = _run(_shard_inputs(inputs))
    out = np.empty((B, C, H, W, D), np.float32)
    for ci in range(NCORES):
        sl = slice(HS * ci, HS * (ci + 1))
        out[:, :, sl] = res.results[ci]["out"].reshape(B, C, HS, W, D)
    return out



# revision 73
# speedup vs baseline: 32.6439x; 1.0057x over previous
"""Trainium2 Bass kernel for nn_DecoderCrossAttention.

Reference computation (per voxel v, batch b):
    q = Wq x_v + bq                        (x = decoder_features, [C])
    k_j = Wk y_jv + bk, v_j = Wv y_jv + bv (y = skip features, COND=4 frames)
    s_j[h] = <q_h, k_jh> / sqrt(DH)        (NH=8 heads of DH=16)
    attn = softmax_j(s)                    (over the 4 conditioning frames)
    o = Wo (sum_j attn_j * v_j) + bo + x_v
    out = GroupNorm8(o) * gamma + beta     (stats over (C/G, H, W, D) per batch)

Strategy (8 NeuronCores, data-parallel over H):
  * Each core gets H-slice of 4 planes: 2*4*32*32 = 8192 voxels.
  * Feature-major layout [C=128 partitions, voxels in free dim], 512-voxel tiles.
  * x/y/Wq/Wk/Wv in bf16 (host-cast, host-transposed weights): halves input
    DMA; rel-err gate is 2e-2, measured ~2.4e-3.
  * Per-head score reduction (sum over the 16 channels of a head) and the
    softmax broadcast (8 head rows -> 128 channels) are PE matmuls against
    0/1 masks built in-kernel with iota+compare.
  * Softmax over only 4 logits, inputs are bounded => no max subtraction.
  * E~ = exp(s)*recip(Z): exp on Act, recip on DVE, the product on Pool.
  * qk = k*q via a stride-0 broadcast AP of q over the 2-cond pair (one
    PSUM operand only - NCC_IBVF027).  bk is dropped entirely: within a
    head it shifts all 4 logits by the same q.bk, which softmax cancels
    exactly.
  * attn*V products (per-cond, through two ping-ponged 1-bank PSUM pools
    so the PE broadcast hides under the previous cond's DVE multiply) feed
    4 accumulating output-projection matmuls, the residual rides in as a
    5th identity matmul, and the Act engine evacuates psO with
    bias+per-channel sums (accum_out); ssq via Act Square.  (GPSIMD
    cannot access PSUM on HW, so every PSUM evacuation stays on Act/DVE.)
  * GroupNorm is global: per-channel sum/sumsq AllGather (15us fixed vs
    28us for AllReduce in the cost model) + local reduce; a
    zero-contribution stt gates the batch-0 finalization into the batch-1
    AllGather window.  Group stats are reduced+broadcast+meaned in ONE
    pre-scaled [C,C] group-mask matmul, variance via sqrt(scale=-1,
    bias=eps), and the rescale runs on tile-PAIRS, all on GpSimd (Act
    serializes the rep-boundary refill and DVE paces the loop; Pool
    idles in both), with paired [C, 2*NT] stores.
  * Both per-tile stats live in ONE [C, 2*B*NTILES] tile (sums | ssqs)
    so each batch's AllGather input is a single strided TensorReduce.
  * THE key multi-rep optimization: each rep's batch-1 finalization
    (gather-DMA + stats chain + rescale + stores, all gated on that
    rep's 15us AllGather) is EMITTED INTO THE NEXT REP'S instruction
    stream, after its tile 5.  The engine SEQs are in-order, so a wait
    placed at the end of rep r would block every engine's rep r+1 work
    behind the collective; deferred emission lets rep r+1's front run
    during rep r's AllGather window, cutting the marginal per-rep time
    from 134.3us to 109.2us in the TimelineSim cost model.
  * Constants are packed into 3 DMAs (wqkv, wo, vec6) and y is fetched
    per-cond with a 2-tile prefetch: first matmul fires at ~5us.

The walrus build here accepts only ONE sync wait per instruction; Tile
attaches many.  split_waits() hoists extras onto standalone EventSemaphore
instructions post-scheduling.
"""

import sys

if "/opt/trn_rl_repo" not in sys.path:
    sys.path.insert(0, "/opt/trn_rl_repo")

import numpy as np

B, COND, C, H, W, D = 2, 4, 128, 32, 32, 32
NH, DH, G = 8, 16, 8
EPS = 1e-5
NCORES = 8
HS = H // NCORES          # 4 H-planes per core
NVOX = HS * W * D         # 4096 voxels per batch per core
NT = 512                  # voxels per tile
NTILES = NVOX // NT       # 8 tiles per batch
N_GROUP = (C // G) * H * W * D   # elements per (batch, group) for GN stats

_CACHE = {}


def _split_waits(nc):
    """Hoist extra sync waits onto standalone EventSemaphore instructions."""
    from concourse import mybir
    import bass_rust

    n_split = 0
    for func in nc.m.functions:
        for blk in func.blocks:
            new_list = []
            changed = False
            for inst in blk.instructions:
                si = inst.sync_info
                waits = list(si.on_wait) if si is not None else []
                if len(waits) > 1:
                    changed = True
                    for w in waits[:-1]:
                        ev = mybir.InstEventSemaphore(
                            name=f"wsplit-{nc.next_id()}", ins=[], outs=[]
                        )
                        ev.engine = inst.engine
                        ev.sync_info = bass_rust.SyncInfo(on_wait=[w], on_update=[])
                        new_list.append(ev)
                        n_split += 1
                    inst.sync_info = bass_rust.SyncInfo(
                        on_wait=[waits[-1]], on_update=list(si.on_update)
                    )
                new_list.append(inst)
            if changed:
                blk.instructions = new_list
    return n_split


def _build(n_reps=1):
    import concourse.bass as bass
    import concourse.tile as tile
    from concourse import mybir
    from concourse.bass_isa import ReduceOp
    from contextlib import ExitStack

    dt = mybir.dt
    f32 = dt.float32
    f32r = dt.float32r
    i32 = dt.int32
    Alu = mybir.AluOpType
    Act = mybir.ActivationFunctionType
    ts = bass.ts

    bf16 = dt.bfloat16

    nc = bass.Bass("TRN2", target_bir_lowering=False, debug=False,
                   num_devices=NCORES)
    x_io = nc.dram_tensor("x", [B, C, NVOX], bf16, kind="ExternalInput").ap()
    y_io = nc.dram_tensor("y", [B, COND, C, NVOX], bf16, kind="ExternalInput").ap()
    # constants packed into 3 tensors: each dma_start costs ~0.6us of HWDGE
    # issue time, so 10 separate loads would delay the first x/y input DMAs
    wqkv_io = nc.dram_tensor("wqkv", [C, 3 * C], bf16, kind="ExternalInput").ap()
    wo_io = nc.dram_tensor("wo", [C, C], f32r, kind="ExternalInput").ap()
    vec6_io = nc.dram_tensor("vec6", [C, 6], f32, kind="ExternalInput").ap()
    out_io = nc.dram_tensor("out", [B, C, NVOX], f32, kind="ExternalOutput").ap()

    def mm(out, lhsT, rhs, start=True, stop=True):
        nc.tensor.matmul(out, lhsT=lhsT, rhs=rhs, start=start, stop=stop)

    with tile.TileContext(nc) as tc, ExitStack() as ctx:
        # ---------------- constants / weights / masks -------------------
        const = ctx.enter_context(tc.tile_pool(name="const", bufs=1))
        dram = ctx.enter_context(tc.tile_pool(name="dram", bufs=1, space="DRAM"))

        # constants first, on the Activation HWDGE queue, packed: 3 issues
        wqkv = const.tile([C, 3 * C], bf16, tag="wqkv")
        nc.scalar.dma_start(wqkv[:], wqkv_io[:])
        wo_t = const.tile([C, C], f32r, tag="wT_wo")
        nc.scalar.dma_start(wo_t[:], wo_io[:])
        vec6 = const.tile([C, 6], f32, tag="vec6")
        nc.scalar.dma_start(vec6[:], vec6_io[:])
        wT = {"wq": wqkv[:, 0:C], "wk": wqkv[:, C:2 * C],
              "wv": wqkv[:, 2 * C:3 * C], "wo": wo_t[:]}
        vecs = {name: vec6[:, i:i + 1] for i, name in
                enumerate(("bq", "bk", "bv", "bo", "gamma", "beta"))}

        # --- masks via iota + compare (int32), cast to f32
        with tc.tile_pool(name="setup", bufs=1) as setup:
            def icast(dst_ap, src_ap):
                nc.vector.tensor_copy(dst_ap, src_ap)

            # partition-index and free-index helpers
            p128 = setup.tile([C, C], i32, tag="p128")
            nc.gpsimd.iota(p128[:], pattern=[[0, C]], base=0, channel_multiplier=1)
            f128 = setup.tile([C, C], i32, tag="f128")
            nc.gpsimd.iota(f128[:], pattern=[[1, C]], base=0, channel_multiplier=0)
            hc128 = setup.tile([C, C], i32, tag="hc128")
            nc.vector.tensor_scalar(hc128[:], p128[:], 4, None,
                                    Alu.arith_shift_right)
            tmpi = setup.tile([C, C], i32, tag="tmpi")

            # identity [128,128] (bf16, for the residual pass-through matmul)
            ident = const.tile([C, C], bf16, tag="ident")
            nc.vector.tensor_tensor(tmpi[:], f128[:], p128[:], Alu.is_equal)
            icast(ident[:], tmpi[:])

            # mask32 [128, 4*32]: col 32j+m ; 1 iff (m - 8j) == c//16
            jm = setup.tile([C, C], i32, tag="jm")
            nc.gpsimd.iota(jm[:].rearrange("p (j m) -> p j m", j=4),
                           pattern=[[-8, 4], [1, 32]], base=0,
                           channel_multiplier=0)
            mask32 = const.tile([C, C], bf16, tag="mask32")
            nc.vector.tensor_tensor(tmpi[:], jm[:], hc128[:], Alu.is_equal)
            icast(mask32[:], tmpi[:])

            # lhsT32 [32,32]: 1 iff p%8 == m%8  (Z replication matmul)
            p32 = setup.tile([32, 32], i32, tag="p32")
            nc.gpsimd.iota(p32[:], pattern=[[0, 32]], base=0, channel_multiplier=1)
            pm32 = setup.tile([32, 32], i32, tag="pm32")
            nc.vector.tensor_scalar(pm32[:], p32[:], 3, 3,
                                    Alu.arith_shift_right, Alu.arith_shift_left)
            t32 = setup.tile([32, 32], i32, tag="t32")
            nc.vector.tensor_tensor(t32[:], p32[:], pm32[:], Alu.subtract)
            fm32 = setup.tile([32, 32], i32, tag="fm32")
            nc.gpsimd.iota(fm32[:].rearrange("p (j m) -> p j m", j=4),
                           pattern=[[0, 4], [1, 8]], base=0, channel_multiplier=0)
            e32 = setup.tile([32, 32], i32, tag="e32")
            nc.vector.tensor_tensor(e32[:], fm32[:], t32[:], Alu.is_equal)
            lhsT32 = const.tile([32, 32], f32r, tag="lhsT32")
            icast(lhsT32[:], e32[:])

            # maskb [32, 4*128]: col 128j+c ; 1 iff (p - 8j) == c//16
            pj = setup.tile([32, 4 * C], i32, tag="pj")
            nc.gpsimd.iota(pj[:].rearrange("p (j c) -> p j c", j=4),
                           pattern=[[-8, 4], [0, C]], base=0,
                           channel_multiplier=1)
            fc = setup.tile([32, 4 * C], i32, tag="fc")
            nc.gpsimd.iota(fc[:].rearrange("p (j c) -> p j c", j=4),
                           pattern=[[0, 4], [1, C]], base=0, channel_multiplier=0)
            nc.vector.tensor_scalar(fc[:], fc[:], 4, None, Alu.arith_shift_right)
            eb = setup.tile([32, 4 * C], i32, tag="eb")
            nc.vector.tensor_tensor(eb[:], pj[:], fc[:], Alu.is_equal)
            maskb = const.tile([32, 4 * C], bf16, tag="maskb")
            icast(maskb[:], eb[:])

            # ggmask [128, 128]: 1/N_GROUP iff p//16 == c//16  (GN group sum,
            # fused reduce+broadcast+mean: psGB = ggmask.T @ stats gives the
            # group means [mean, E[x^2]] directly at channel layout)
            fg = setup.tile([C, C], i32, tag="fg")
            nc.vector.tensor_scalar(fg[:], f128[:], 4, None,
                                    Alu.arith_shift_right)
            egg = setup.tile([C, C], i32, tag="egg")
            nc.vector.tensor_tensor(egg[:], fg[:], hc128[:], Alu.is_equal)
            ggmask = const.tile([C, C], f32, tag="ggmask")
            icast(ggmask[:], egg[:])
            nc.vector.tensor_scalar(ggmask[:], ggmask[:], 1.0 / N_GROUP, None,
                                    Alu.mult)
            eps_t = const.tile([C, 1], f32, tag="eps_t")
            nc.vector.memset(eps_t[:], EPS)

        # ---------------- main pipeline ---------------------------------
        per_rep_pools = dict(
            xres=ctx.enter_context(tc.tile_pool(name="xres", bufs=2)),
            ypool=ctx.enter_context(tc.tile_pool(name="ypool", bufs=2)),
            sb=ctx.enter_context(tc.tile_pool(name="sb", bufs=2)),
            big=ctx.enter_context(tc.tile_pool(name="bigsb", bufs=2)),
            # bufs=2 so rep r+1's accumulation/stats don't serialize behind
            # rep r's finalization (cross-rep overlap; no-op for n_reps=1)
            opool=ctx.enter_context(tc.tile_pool(name="opool", bufs=2)),
            stats=ctx.enter_context(tc.tile_pool(name="stats", bufs=2)),
            ps_kb=ctx.enter_context(tc.tile_pool(name="ps_kb", bufs=1, space="PSUM")),
            ps_bb1=ctx.enter_context(tc.tile_pool(name="ps_bb1", bufs=1, space="PSUM")),
            ps_bb2=ctx.enter_context(tc.tile_pool(name="ps_bb2", bufs=1, space="PSUM")),
            ps_q=ctx.enter_context(tc.tile_pool(name="ps_q", bufs=1, space="PSUM")),
            ps_v=ctx.enter_context(tc.tile_pool(name="ps_v", bufs=1, space="PSUM")),
            ps_s=ctx.enter_context(tc.tile_pool(name="ps_s", bufs=1, space="PSUM")),
            ps_o=ctx.enter_context(tc.tile_pool(name="ps_o", bufs=1, space="PSUM")),
        )

        def emit_rep(rep, deferred):
            """Emit one rep; returns this rep's deferred b1 finalization.

            deferred: previous rep's b1 finalization, emitted into THIS
            rep's stream after tile 3 so its AllGather wait is already
            satisfied when each engine SEQ reaches it.  A nested function
            so each rep's closures bind their own tiles (the rep loop
            would otherwise rebind shared locals under the deferred call).
            """
            p = per_rep_pools
            out_acc = p["opool"].tile([C, B * NVOX], f32, tag="out_acc")
            # per-tile stats, one tile: col b*NTILES+t = sums, col
            # B*NTILES + b*NTILES + t = sums-of-squares (so each batch's
            # sums and ssqs reduce in ONE strided TensorReduce)
            BN = B * NTILES
            st2 = p["stats"].tile([C, 2 * BN], f32, tag="st2")
            dump = p["stats"].tile([C, NT], f32, tag="dump")

            tiles = [(b, t) for b in range(B) for t in range(NTILES)]
            xres_b = {}
            ychunk_state = {}
            fstate = {}
            sstate = {}

            def load_x(b):
                if b in xres_b:
                    return
                xr = p["xres"].tile([C, NVOX], bf16, tag="xres")
                nc.sync.dma_start(xr[:], x_io[b])
                xres_b[b] = xr

            def load_ychunk(b, ci):
                if (b, ci) in ychunk_state:
                    return
                yc = p["ypool"].tile([C, COND * 4 * NT], bf16, tag="ychunk")
                ysrc = y_io[b].rearrange("j c v -> c j v")
                # per-cond DMAs: K-proj for cond j only waits on its slice
                for j in range(COND):
                    nc.sync.dma_start(
                        yc[:, j * 4 * NT: (j + 1) * 4 * NT],
                        ysrc[:, bass.ts(j, 1), bass.ts(ci, 4 * NT)]
                        .rearrange("p j v -> p (j v)"),
                    )
                ychunk_state[(b, ci)] = yc

            def front1(k):
                """DMAs, Q proj, K h0 proj, V proj+copies."""
                b, t = tiles[k]
                load_x(b)
                load_ychunk(b, t // 4)
                ychunk = ychunk_state[(b, t // 4)]
                yj_of = lambda j: ychunk[:, j * 4 * NT + (t % 4) * NT:
                                         j * 4 * NT + (t % 4 + 1) * NT]
                xt = xres_b[b][:, ts(t, NT)]
                psQ = p["ps_q"].tile([C, NT], f32, tag="psq")
                mm(psQ[:], wT["wq"], xt)
                qsb = p["sb"].tile([C, NT], bf16, tag="qsb")
                nc.scalar.activation(qsb[:], psQ[:], Act.Identity,
                                     bias=vecs["bq"])
                psKB0 = p["ps_kb"].tile([C, 2 * NT], f32, tag="kb")
                for j in (0, 1):
                    mm(psKB0[:, ts(j, NT)], wT["wk"], yj_of(j))
                vbig = p["big"].tile([C, COND * NT], f32, tag="vbig")
                for j in range(COND):
                    psV = p["ps_v"].tile([C, NT], f32, tag="psv")
                    mm(psV[:], wT["wv"], yj_of(j))
                    nc.scalar.activation(vbig[:, ts(j, NT)], psV[:],
                                         Act.Identity, bias=vecs["bv"])
                qkbig = p["big"].tile([C, COND * NT], bf16, tag="qkbig")
                psS = p["ps_s"].tile([32, NT], f32, tag="pss")
                fstate[k] = (psS, vbig, xt, qsb, qkbig, yj_of, psKB0)
                # prefetch the next y chunk / next batch's inputs during the
                # current chunk's idle DMA time (avoids a burst at t%4==0 and
                # at the batch boundary, where stores also compete)
                if t % 4 == 2:
                    if t // 4 + 1 < NTILES // 4:
                        load_ychunk(b, t // 4 + 1)
                    elif b + 1 < B:
                        load_x(b + 1)
                        load_ychunk(b + 1, 0)

            def front_qk(k, h):
                """QK mul half h + its score matmuls (+ K h1 projections).

                qk = k * q via a stride-0 broadcast of qsb over the 2-cond
                pair (one PSUM operand only: NCC_IBVF027).  bk is dropped
                entirely: within a head it shifts all 4 logits by the same
                q.bk, which softmax cancels exactly.
                """
                psS, vbig, xt, qsb, qkbig, yj_of, psKB = fstate[k]
                qrep = qsb[:].unsqueeze(1).broadcast_to([C, 2, NT])
                nc.vector.tensor_tensor(
                    qkbig[:, ts(h, 2 * NT)].rearrange("p (j v) -> p j v", j=2),
                    psKB[:].rearrange("p (j v) -> p j v", j=2),
                    qrep, Alu.mult)
                for j in (2 * h, 2 * h + 1):
                    mm(psS[:], mask32[:, ts(j, 32)], qkbig[:, ts(j, NT)],
                       start=(j == 0), stop=(j == COND - 1))
                if h == 0:
                    psKB1 = p["ps_kb"].tile([C, 2 * NT], f32, tag="kb")
                    for j in (2, 3):
                        mm(psKB1[:, ts(j - 2, NT)], wT["wk"], yj_of(j))
                    fstate[k] = (psS, vbig, xt, qsb, qkbig, yj_of, psKB1)

            def soft(k):
                """exp, Z-matmul, reciprocal, E~ = E * (1/Z)."""
                b, t = tiles[k]
                psS, vbig, xt, qsb, qkbig, yj_of, _ = fstate.pop(k)
                esb = p["sb"].tile([32, NT], f32r, tag="esb")
                nc.scalar.activation(esb[:], psS[:], Act.Exp, scale=0.25)
                # psZ lives in the ps_s pool: its WAR (next tile's scores)
                # sits later in the chain than ps_q's (next tile's Q-proj).
                psZ = p["ps_s"].tile([32, NT], f32, tag="pss")
                mm(psZ[:], lhsT32[:], esb[:])
                rsb = p["sb"].tile([32, NT], f32, tag="rsb")
                nc.vector.reciprocal(rsb[:], psZ[:])
                etsb = p["sb"].tile([32, NT], bf16, tag="etsb")
                nc.gpsimd.tensor_tensor(etsb[:], esb[:].bitcast(f32),
                                        rsb[:], Alu.mult)
                sstate[k] = (etsb, vbig, xt)

            def back_cond(k, j, wbig):
                """Broadcast matmul + attn*V multiply for one cond.

                Per-cond with two ping-ponged 1-bank PSUM pools: bcast j+2
                only waits on avm j, so the PE bcast hides under the DVE
                multiply of the previous cond."""
                etsb, vbig, xt = sstate[k]
                pool = p["ps_bb1"] if j % 2 == 0 else p["ps_bb2"]
                psB = pool.tile([C, NT], f32, tag="bb")
                mm(psB[:], maskb[:, ts(j, C)], etsb[:])
                nc.vector.tensor_tensor(
                    wbig[:, ts(j, NT)], psB[:],
                    vbig[:, ts(j, NT)], Alu.mult)

            def back_out(k, wbig):
                b, t = tiles[k]
                col = b * NTILES + t
                etsb, vbig, xt = sstate.pop(k)
                psO = p["ps_o"].tile([C, NT], f32, tag="pso")
                for j in range(COND):
                    mm(psO[:], wT["wo"], wbig[:, ts(j, NT)],
                       start=(j == 0), stop=False)
                # residual folded into the accumulation group: psO += I @ x
                mm(psO[:], ident[:], xt, start=False, stop=True)
                outt = out_acc[:, col * NT: (col + 1) * NT]
                nc.scalar.activation(
                    outt, psO[:], Act.Identity, bias=vecs["bo"],
                    accum_out=st2[:, col: col + 1])
                nc.scalar.activation(
                    dump[:], outt, Act.Square,
                    accum_out=st2[:, BN + col: BN + col + 1])

            cc_state = {}
            cc_sb = {}

            def gn_pre(b):
                """Reduce per-channel stats and launch the AllGather."""
                ccsb = p["stats"].tile([C, 2], f32, tag=f"ccsb{b}")
                # ONE strided reduce over [C, (2 stats, NTILES)] -> [C, 2]
                nc.vector.reduce_sum(
                    ccsb[:],
                    st2[:].rearrange("p (s x) -> p s x", s=2)
                    [:, :, b * NTILES:(b + 1) * NTILES],
                    axis=mybir.AxisListType.X)
                cc_in = dram.tile([C, 2], f32, tag=f"cc_in{b}")
                cc_out = dram.tile([NCORES, C, 2], f32, tag=f"cc_out{b}")
                # Act HWDGE queue: must not sit behind bulk stores on SP
                nc.scalar.dma_start(cc_in[:], ccsb[:])
                nc.gpsimd.collective_compute(
                    "AllGather", Alu.bypass,
                    replica_groups=[list(range(NCORES))],
                    ins=[cc_in.opt()], outs=[cc_out.opt()])
                cc_state[b] = cc_out
                cc_sb[b] = ccsb

            def gn_post(b):
                """Stats -> per-channel affine -> rescale out_acc -> store."""
                cc_out = cc_state.pop(b)
                # gather the 8 cores' [C,2] partials: gs16[:, 0:8]=sums,
                # gs16[:, 8:16]=ssqs (s-major so the reduces are contiguous)
                gs16 = p["stats"].tile([C, 16], f32, tag=f"gs16_{b}")
                nc.sync.dma_start(
                    gs16[:].rearrange("p (s n) -> p s n", n=NCORES),
                    cc_out[:].rearrange("n p s -> p s n"))
                # one reduce over cores -> per-channel sums [C, 2]
                gsb = p["stats"].tile([C, 2], f32, tag=f"gsb{b}")
                nc.vector.reduce_sum(
                    gsb[:], gs16[:].rearrange("p (s n) -> p s n", n=NCORES),
                    axis=mybir.AxisListType.X)
                # fused group reduce+broadcast+mean: psGB = [mean, E[x^2]].
                # The ps_q WAR this creates is harmless: for b=1 this op is
                # emitted after the NEXT rep's tile 3 (deferred), so the
                # next rep's early Q-projections precede it in the stream.
                psGB = p["ps_q"].tile([C, 2], f32, tag="psq")
                nc.tensor.matmul(psGB[:], lhsT=ggmask[:], rhs=gsb[:],
                                 start=True, stop=True)
                msb = p["stats"].tile([C, 2], f32, tag=f"msb{b}")
                nc.vector.tensor_copy(msb[:], psGB[:])
                # negvar = mean^2 - E[x^2]; rstd = 1/sqrt(eps - negvar)
                nvar = p["stats"].tile([C, 1], f32, tag=f"nvar{b}")
                nc.vector.scalar_tensor_tensor(
                    nvar[:], msb[:, 0:1], msb[:, 0:1], msb[:, 1:2],
                    Alu.mult, Alu.subtract)
                sstd = p["stats"].tile([C, 1], f32, tag=f"sstd{b}")
                nc.scalar.activation(sstd[:], nvar[:], Act.Sqrt,
                                     bias=eps_t[:], scale=-1.0)
                rstd = p["stats"].tile([C, 1], f32, tag=f"rstd{b}")
                nc.vector.reciprocal(rstd[:], sstd[:])
                scale_b = p["stats"].tile([C, 1], f32, tag=f"scale{b}")
                nc.vector.tensor_tensor(scale_b[:], rstd[:],
                                        vecs["gamma"], Alu.mult)
                gate = cc_sb.get(1 - b) if b == 0 else None
                if gate is not None:
                    # 0*ccsb(b1) + scale: a zero-contribution data dependency
                    # that stops the scheduler hoisting b0's rescale+stores
                    # out of the b1 AllGather window they are meant to fill
                    gated = p["stats"].tile([C, 1], f32, tag=f"gsc{b}")
                    nc.vector.scalar_tensor_tensor(
                        gated[:], gate[:, 0:1], 0.0, scale_b[:],
                        Alu.mult, Alu.add)
                    scale_b = gated
                negb_b = p["stats"].tile([C, 2], f32, tag=f"negb{b}")
                nc.vector.scalar_tensor_tensor(
                    negb_b[:, 0:1], msb[:, 0:1], scale_b[:],
                    vecs["beta"], Alu.mult, Alu.subtract)
                nc.vector.tensor_scalar(negb_b[:, 1:2], negb_b[:, 0:1],
                                        -1.0, None, Alu.mult)
                # rescale tile-PAIRS split over three engines (DVE 2x all-
                # SBUF mode / Act / Pool), storing each pair once rescaled
                for tp in range(NTILES // 2):
                    src = out_acc[:, (b * NTILES + 2 * tp) * NT:
                                  (b * NTILES + 2 * tp + 2) * NT]
                    # all rescale pairs on GpSimd: Act serializes the rep
                    # boundary and DVE paces the loop; Pool idles in both
                    nc.gpsimd.tensor_scalar(
                        src, src,
                        scale_b[:], negb_b[:, 0:1], Alu.mult, Alu.subtract)
                    nc.sync.dma_start(out_io[b][:, ts(tp, 2 * NT)], src)

            NK = len(tiles)
            for k in range(NK + 2):
                if k == 6 and deferred is not None:
                    deferred()
                    deferred = None
                if 1 <= k <= NK:
                    soft(k - 1)
                if k >= 2:
                    wbig = p["big"].tile([C, COND * NT], f32r, tag="qkbig")
                    back_cond(k - 2, 0, wbig)
                    back_cond(k - 2, 1, wbig)
                if k < NK:
                    front1(k)
                    front_qk(k, 0)
                if k >= 2:
                    back_cond(k - 2, 2, wbig)
                    back_cond(k - 2, 3, wbig)
                if k < NK:
                    front_qk(k, 1)
                if k >= 2:
                    back_out(k - 2, wbig)
                    bdone, tdone = tiles[k - 2]
                    if tdone == NTILES - 1:
                        gn_pre(bdone)
            # gn_post(0) emits now: it depends only on the (long-finished)
            # b0 AllGather and its gated rescale+stores fill the b1
            # AllGather latency.  gn_post(1) is deferred into the NEXT
            # rep's stream (emitted after its tile 3) so the b1 AllGather
            # wait never blocks an engine SEQ ahead of next-rep work.
            gn_post(0)
            return lambda: gn_post(1)

        pending = None
        for rep in range(n_reps):
            pending = emit_rep(rep, pending)
        pending()


    _split_waits(nc)
    return nc


def _shard_inputs(inputs):
    import ml_dtypes
    bf16 = ml_dtypes.bfloat16
    x = np.asarray(inputs["decoder_features"], np.float32).astype(bf16)
    y = np.asarray(inputs["skip_connection_features"], np.float32).astype(bf16)

    def wT(name, dtype):
        w = np.asarray(inputs[name], np.float32)
        return np.ascontiguousarray(w.T).astype(dtype)

    wqkv = np.concatenate([wT("w_q", bf16), wT("w_k", bf16),
                           wT("w_v", bf16)], axis=1)
    vec6 = np.stack([np.asarray(inputs[n], np.float32) for n in
                     ("b_q", "b_k", "b_v", "b_o", "gn_gamma", "gn_beta")],
                    axis=1)
    base = {
        "wqkv": np.ascontiguousarray(wqkv),
        "wo": wT("w_o", np.float32),
        "vec6": np.ascontiguousarray(vec6),
    }
    in_maps = []
    for ci in range(NCORES):
        sl = slice(HS * ci, HS * (ci + 1))
        im = dict(base)
        im["x"] = np.ascontiguousarray(x[:, :, sl]).reshape(B, C, NVOX)
        im["y"] = np.ascontiguousarray(y[:, :, :, sl]).reshape(B, COND, C, NVOX)
        in_maps.append(im)
    return in_maps


class _Runner:
    """Persistent PJRT runner: trace/compile once, execute many times.

    Mirrors concourse.bass2jax.run_bass_via_pjrt's multi-core branch but
    keeps the jitted shard_map callable alive so repeat calls skip
    re-tracing and NEFF recompilation.
    """

    def __init__(self, nc, donate=True):
        import jax
        from jax.sharding import Mesh, PartitionSpec
        from jax.experimental.shard_map import shard_map
        from concourse import bass2jax, mybir

        bass2jax.install_neuronx_cc_hook()
        assert nc.dbg_addr is None
        partition_name = (nc.partition_id_tensor.name
                          if nc.partition_id_tensor else None)
        in_names, out_names, out_avals, zero_outs = [], [], [], []
        for alloc in nc.m.functions[0].allocations:
            if not isinstance(alloc, mybir.MemoryLocationSet):
                continue
            name = alloc.memorylocations[0].name
            if alloc.kind == "ExternalInput":
                if name != partition_name:
                    in_names.append(name)
            elif alloc.kind == "ExternalOutput":
                out_names.append(name)
                shape = tuple(alloc.tensor_shape)
                dtype = mybir.dt.np(alloc.dtype)
                out_avals.append(jax.core.ShapedArray(shape, dtype))
                zero_outs.append(np.zeros(shape, dtype))
        n_params = len(in_names)
        n_outs = len(out_avals)
        in_names.extend(out_names)
        if partition_name is not None:
            in_names.append(partition_name)
        donate_idx = tuple(range(n_params, n_params + n_outs)) if donate else ()

        def _body(*args):
            operands = list(args)
            if partition_name is not None:
                operands.append(bass2jax.partition_id_tensor())
            outs = bass2jax._bass_exec_p.bind(
                *operands,
                out_avals=tuple(out_avals),
                in_names=tuple(in_names),
                out_names=tuple(out_names),
                lowering_input_output_aliases=(),
                sim_require_finite=True,
                sim_require_nnan=True,
                nc=nc,
            )
            return tuple(outs)

        devices = jax.devices()[:NCORES]
        mesh = Mesh(np.asarray(devices), ("core",))
        in_specs = (PartitionSpec("core"),) * (n_params + n_outs)
        out_specs = (PartitionSpec("core"),) * n_outs
        self._fn = jax.jit(
            shard_map(_body, mesh=mesh, in_specs=in_specs,
                      out_specs=out_specs, check_rep=False),
            donate_argnums=donate_idx, keep_unused=True)
        self._in_names = in_names[:n_params]
        self._out_names = out_names
        self._out_avals = out_avals
        self._zero_outs = zero_outs
        self._jax = jax

    def __call__(self, in_maps):
        concat_in = [
            np.concatenate([np.asarray(m[name]) for m in in_maps], axis=0)
            for name in self._in_names
        ]
        concat_zeros = [
            np.zeros((NCORES * z.shape[0], *z.shape[1:]), z.dtype)
            for z in self._zero_outs
        ]
        out_arrs = self._fn(*concat_in, *concat_zeros)
        out_arrs = self._jax.block_until_ready(out_arrs)
        return [
            {
                name: np.asarray(out_arrs[i]).reshape(
                    NCORES, *self._out_avals[i].shape)[c]
                for i, name in enumerate(self._out_names)
            }
            for c in range(NCORES)
        ]


class _Results:
    def __init__(self, results):
        self.results = results


def _get_runner(n_reps=1, donate=True):
    key = (n_reps, donate)
    if key not in _CACHE:
        _CACHE[key] = _Runner(_build(n_reps), donate=donate)
    return _CACHE[key]


def _run(in_maps, n_reps=1):
    return _Results(_get_runner(n_reps)(in_maps))


def kernel(**inputs) -> np.ndarray:
    res = _run(_shard_inputs(inputs))
    out = np.empty((B, C, H, W, D), np.float32)
    for ci in range(NCORES):
        sl = slice(HS * ci, HS * (ci + 1))
        out[:, :, sl] = res.results[ci]["out"].reshape(B, C, HS, W, D)
    return out

